# revision 2
# baseline (speedup 1.0000x reference)
"""DSGIAT GraphBranch: full-pipeline Bass kernel for 8 trn2 NeuronCores.

Node-sharded (3840 padded nodes/core).  Per core:
  GEMM (own shard) -> AllGather H table -> dst-sharded edge aggregation via
  dma_gather + indicator-matmul segment-sum + dma_scatter_add -> softmax
  division -> label-prop (separable sym-norm, indicator matmuls) with
  AllGather between steps -> pooling partials.  Host: preprocessing,
  x-pooling, final MLP.
"""
import numpy as np
import ml_dtypes
from contextlib import ExitStack

BF16 = ml_dtypes.bfloat16

# ---------------- problem constants ----------------
N = 30000
N_CORES = 8
SH_REAL = 3750           # real nodes per core
SH = 3840                # padded shard (30 * 128)
NP = SH * N_CORES        # 30720
LT = 30                  # local node tiles
IN_CH = 256
OUT1 = 512
HEADS = 4
HID = 128
COLS = 640               # H table width: 0:512 h | 512:516 es | 516:520 ed | pad
NGR = 64
EPS = 1e-16
NEG = 0.2
DUMP = SH                # scatter dump row
NLOC_PAD = SH + 64       # OUT tables rows (incl dump region)

CONV_CHUNKS = 32         # conv edge chunks of 2048 per core
LP_CHUNKS = 15
CHUNK_C = 2048
CHUNK_L = 4096
CONV_E = CONV_CHUNKS * CHUNK_C   # 65536 slots
LP_E = LP_CHUNKS * CHUNK_L       # 61440 slots
CONV_T = CONV_E // 128         # 512 tiles
LP_T = LP_E // 128             # 480 tiles

_cached = {}


# ---------------- host preprocessing ----------------

def _remap(n):
    """original node id -> padded id"""
    return (n // SH_REAL) * SH + (n % SH_REAL)


def _wrap_gidx(idx, nslots):
    """global gather indices -> [128, nslots//16] int16 (wrapped, replicated)."""
    a = np.zeros(nslots, dtype=np.int16)
    a[: len(idx)] = idx.astype(np.int16)
    w = a.reshape(nslots // 16, 16).T
    return np.ascontiguousarray(np.tile(w, (8, 1)))


def _edge_meta(src_r, dst_r, nslots):
    """Per-core edge metadata for dst-sharded aggregation.

    src_r/dst_r: remapped global ids, globally sorted by dst_r.
    Returns per-core dicts + global maxk per tile.
    """
    ntiles = nslots // 128
    cores = []
    for c in range(N_CORES):
        lo, hi = c * SH, (c + 1) * SH
        m = (dst_r >= lo) & (dst_r < hi)
        s = src_r[m]
        d = dst_r[m] - lo
        ne = len(s)
        assert ne <= nslots, f"core {c}: {ne} > {nslots}"
        spad = np.zeros(nslots, dtype=np.int64)
        spad[:ne] = s
        dpad = np.full(nslots, 1 << 20, dtype=np.int64)  # pad sentinel
        dpad[:ne] = d
        pos = np.arange(nslots)
        newseg = np.ones(nslots, dtype=bool)
        newseg[1:] = (dpad[1:] != dpad[:-1])
        newseg |= (pos % 128 == 0)
        segg = np.cumsum(newseg) - 1
        tile_id = pos // 128
        tile_start_seg = segg[pos - (pos % 128)]
        segoff = segg - tile_start_seg          # [nslots]
        k = segoff[pos % 128 == 127] + 1        # per-tile seg count [ntiles]
        # sidx values: for each (tile, rank) the local dst (or DUMP for pad)
        sidx = np.full((ntiles, 128), -1, dtype=np.int64)
        first = newseg
        tv = tile_id[first]
        rv = segoff[first]
        dv = dpad[first]
        dv = np.where(dv == (1 << 20), DUMP, dv)
        sidx[tv, rv] = dv
        dglob = np.zeros(nslots, dtype=np.int64)
        dglob[:ne] = dst_r[m]
        cores.append(dict(s=spad, d=dglob, segoff=segoff.astype(np.float32),
                          sidx=sidx, k=k))
    maxk = np.max(np.stack([c["k"] for c in cores]), axis=0)  # [ntiles]
    for c in cores:
        sidx = c["sidx"]
        k = c["k"]
        for t in range(ntiles):
            if k[t] < maxk[t]:
                sidx[t, k[t]:maxk[t]] = DUMP
        # wrap: [ntiles, 128] -> [128, ntiles*8]
        w = sidx.astype(np.int16).reshape(ntiles, 8, 16).transpose(2, 0, 1)
        w = w.reshape(16, ntiles * 8)
        c["sidx_w"] = np.ascontiguousarray(np.tile(w, (8, 1)))
        # segoff wrapped: edge i of tile t -> partition i, col t
        c["segoff_w"] = np.ascontiguousarray(
            c["segoff"].reshape(ntiles, 128).T)
        c["gidx_w"] = _wrap_gidx(c["s"], nslots)
        c["didx_w"] = _wrap_gidx(c["d"], nslots)
    return cores, maxk


def _preprocess(x, edge_index, batch):
    src = np.asarray(edge_index[0], dtype=np.int64)
    dst = np.asarray(edge_index[1], dtype=np.int64)
    batch = np.asarray(batch, dtype=np.int64)

    deg = np.bincount(dst, minlength=N).astype(np.float32)
    dis = np.where(deg > 0, 1.0 / np.sqrt(np.maximum(deg, 1.0)), 0.0)

    src_r = _remap(src)
    dst_r = _remap(dst)
    loop = _remap(np.arange(N, dtype=np.int64))

    # conv edges (raw + self loops), sorted by dst
    cs = np.concatenate([src_r, loop])
    cd = np.concatenate([dst_r, loop])
    o = np.argsort(cd, kind="stable")
    conv_cores, conv_maxk = _edge_meta(cs[o], cd[o], CONV_E)

    # LP edges (raw), sorted by dst
    o2 = np.argsort(dst_r, kind="stable")
    lp_cores, lp_maxk = _edge_meta(src_r[o2], dst_r[o2], LP_E)

    # per-core node-local tables [128, 30]: local node l = t*128 + p
    dis_pad = np.zeros(NP, dtype=np.float32)
    dis_pad[_remap(np.arange(N))] = dis
    batch_pad = np.full(NP, -1.0, dtype=np.float32)
    batch_pad[_remap(np.arange(N))] = batch.astype(np.float32)

    per_core = []
    for c in range(N_CORES):
        dl = dis_pad[c * SH:(c + 1) * SH].reshape(LT, 128).T  # [128, 30]
        bl = batch_pad[c * SH:(c + 1) * SH].reshape(LT, 128).T
        per_core.append(dict(
            dis=np.ascontiguousarray(dl),
            dish=np.ascontiguousarray(dl * 0.5),
            batch=np.ascontiguousarray(bl),
            conv=conv_cores[c], lp=lp_cores[c]))

    xp = np.zeros((NP, IN_CH), dtype=np.float32)
    xp[_remap(np.arange(N))] = np.asarray(x, dtype=np.float32)
    return per_core, conv_maxk, lp_maxk, xp, dis


def _fold_w(W, a_src, a_dst):
    """[Fi, 512] + [4,128]x2 -> [Fi, 520] bf16 augmented weight."""
    W = np.asarray(W, np.float32)
    wes = np.stack([W[:, h * HID:(h + 1) * HID] @ np.asarray(a_src, np.float32)[h]
                    for h in range(HEADS)], axis=1)
    wed = np.stack([W[:, h * HID:(h + 1) * HID] @ np.asarray(a_dst, np.float32)[h]
                    for h in range(HEADS)], axis=1)
    return np.concatenate([W, wes, wed], axis=1).astype(BF16)


# ---------------- device program ----------------

def _build(conv_maxk, lp_maxk, phases=10):
    import os
    import concourse.tile as tile
    from concourse import bacc, mybir
    no_scat = os.environ.get("NO_SCAT", "0") == "1"
    no_gath = os.environ.get("NO_GATH", "0") == "1"
    max_ch = int(os.environ.get("MAX_CH", "9999"))

    f32 = mybir.dt.float32
    bf16 = mybir.dt.bfloat16
    i16 = mybir.dt.int16
    i32 = mybir.dt.int32
    AG = "AllGather"
    BYP = mybir.AluOpType.bypass
    RG = [list(range(N_CORES))]

    nc = bacc.Bacc("TRN2", target_bir_lowering=False, debug=False,
                   num_devices=N_CORES, dynamic_dma_scratch_size=32768)

    # ---- inputs ----
    xTs = nc.dram_tensor("xTs", [IN_CH, SH], bf16, kind="ExternalInput")
    W1 = nc.dram_tensor("W1", [IN_CH, 520], bf16, kind="ExternalInput")
    W2 = nc.dram_tensor("W2", [OUT1, 520], bf16, kind="ExternalInput")
    bias1 = nc.dram_tensor("bias1", [128, 512], f32, kind="ExternalInput")
    bias2 = nc.dram_tensor("bias2", [128, 512], f32, kind="ExternalInput")
    gixc = nc.dram_tensor("gixc", [128, CONV_E // 16], i16, kind="ExternalInput")
    gixd = nc.dram_tensor("gixd", [128, CONV_E // 16], i16, kind="ExternalInput")
    sixc = nc.dram_tensor("sixc", [128, CONV_T * 8], i16, kind="ExternalInput")
    sofc = nc.dram_tensor("sofc", [128, CONV_T], f32, kind="ExternalInput")
    gixl = nc.dram_tensor("gixl", [128, LP_E // 16], i16, kind="ExternalInput")
    sixl = nc.dram_tensor("sixl", [128, LP_T * 8], i16, kind="ExternalInput")
    sofl = nc.dram_tensor("sofl", [128, LP_T], f32, kind="ExternalInput")
    disd = nc.dram_tensor("disd", [128, LT], f32, kind="ExternalInput")
    dishd = nc.dram_tensor("dishd", [128, LT], f32, kind="ExternalInput")
    batchd = nc.dram_tensor("batchd", [128, LT], f32, kind="ExternalInput")

    # ---- internal DRAM ----
    H1ps = nc.dram_tensor("H1ps", [SH, COLS], bf16)
    H1p = nc.dram_tensor("H1p", [NP, COLS], bf16)
    H2ps = nc.dram_tensor("H2ps", [SH, COLS], bf16)
    H2p = nc.dram_tensor("H2p", [NP, COLS], bf16)
    OUTC1 = nc.dram_tensor("OUTC1", [NLOC_PAD, COLS], bf16)
    OUTC2 = nc.dram_tensor("OUTC2", [NLOC_PAD, COLS], bf16)
    y0s = nc.dram_tensor("y0s", [SH, 512], bf16)
    z0s = nc.dram_tensor("z0s", [SH, 512], bf16)
    y2s = nc.dram_tensor("y2s", [SH, 512], bf16)
    z2s = nc.dram_tensor("z2s", [SH, 512], bf16)
    shp = [nc.dram_tensor(f"sh{i}", [SH, 512], bf16) for i in range(4)]
    Yt = [nc.dram_tensor(f"Y{i}", [NP, 512], bf16) for i in range(4)]
    OUTL = [nc.dram_tensor(f"OUTL{i}", [NLOC_PAD, 512], bf16) for i in range(4)]
    pooled = nc.dram_tensor("pooled", [64, 1024], f32, kind="ExternalOutput")
    dbg = os.environ.get("DBG", "0") == "1"
    dbg_t = {}
    if dbg:
        for nm, (rr, cc) in dict(
                dH1ps=(SH, COLS), dOUTC1=(NLOC_PAD, COLS), dy0s=(SH, 512),
                dsh0=(SH, 512), dOUTL0=(NLOC_PAD, 512), dsh1=(SH, 512),
                dy2s=(SH, 512), dH2ps=(SH, COLS), dOUTC2=(NLOC_PAD, COLS),
                dz0s=(SH, 512), dz2s=(SH, 512)).items():
            dbg_t[nm] = nc.dram_tensor(nm, [rr, cc], bf16,
                                       kind="ExternalOutput")

    with tile.TileContext(nc) as tc, ExitStack() as ctx:
        st = ctx.enter_context(tc.tile_pool(name="st", bufs=1))
        mybir_ = mybir

        # ---------- static SBUF ----------
        iota_i = st.tile([128, 128], i32, tag="iota_i")
        nc.gpsimd.iota(iota_i[:], [[1, 128]], channel_multiplier=0)
        iota_f = st.tile([128, 128], f32, tag="iota_f")
        nc.vector.tensor_copy(iota_f[:], iota_i[:])
        iota_pm = st.tile([128, 128], i32, tag="iota_pm")
        nc.gpsimd.iota(iota_pm[:], [[1, 128]], channel_multiplier=-1)
        ident = st.tile([128, 128], bf16, tag="ident")
        nc.vector.tensor_single_scalar(ident[:], iota_pm[:], 0,
                                       mybir_.AluOpType.is_equal)
        iota64_f = st.tile([128, 64], f32, tag="iota64_f")
        nc.vector.tensor_copy(iota64_f[:], iota_i[:, 0:64])

        def load_const(t_dram, shape, dt, tg):
            t = st.tile(shape, dt, tag=tg, name=tg)
            nc.sync.dma_start(t[:], t_dram[:, :])
            return t

        W1s = st.tile([128, 2, 520], bf16, tag="W1s")
        nc.sync.dma_start(W1s[:, 0, :], W1[0:128, :])
        nc.sync.dma_start(W1s[:, 1, :], W1[128:256, :])
        W2s = st.tile([128, 4, 520], bf16, tag="W2s")
        for k in range(4):
            nc.sync.dma_start(W2s[:, k, :], W2[k * 128:(k + 1) * 128, :])
        b1s = load_const(bias1, [128, 512], f32, "b1s")
        b2s = load_const(bias2, [128, 512], f32, "b2s")
        gixc_s = load_const(gixc, [128, CONV_E // 16], i16, "gixc_s")
        gixd_s = load_const(gixd, [128, CONV_E // 16], i16, "gixd_s")
        sixc_s = load_const(sixc, [128, CONV_T * 8], i16, "sixc_s")
        sofc_s = load_const(sofc, [128, CONV_T], f32, "sofc_s")
        gixl_s = load_const(gixl, [128, LP_E // 16], i16, "gixl_s")
        sixl_s = load_const(sixl, [128, LP_T * 8], i16, "sixl_s")
        sofl_s = load_const(sofl, [128, LP_T], f32, "sofl_s")
        dis_s = load_const(disd, [128, LT], f32, "dis_s")
        dish_s = load_const(dishd, [128, LT], f32, "dish_s")
        batch_s = load_const(batchd, [128, LT], f32, "batch_s")
        y2T_sb = st.tile([128, 4, SH], bf16, tag="y2T_sb")

        zero640 = st.tile([128, COLS], bf16, tag="zero640")
        nc.vector.memset(zero640[:], 0.0)

        # ---------- helpers ----------
        def gemm(pool, psp, src_lhsT, Wsb, nk, out_dram):
            """node-sharded GEMM: out[t*128+p, 0:520] = sum_k lhsT_k.T @ W."""
            for t in range(LT):
                psA = psp.tile([128, 512], f32, space="PSUM", tag="psA",
                               bufs=2)
                psB = psp.tile([128, 8], f32, space="PSUM", tag="psB", bufs=2)
                lts = [src_lhsT(pool, t, k) for k in range(nk)]
                for k in range(nk):
                    nc.tensor.matmul(psA[:], lhsT=lts[k], rhs=Wsb[:, k, 0:512],
                                     start=(k == 0), stop=(k == nk - 1))
                for k in range(nk):
                    nc.tensor.matmul(psB[:], lhsT=lts[k], rhs=Wsb[:, k, 512:520],
                                     start=(k == 0), stop=(k == nk - 1))
                ht = pool.tile([128, COLS], bf16, tag="gemm_out")
                nc.vector.tensor_copy(ht[:, 0:512], psA[:])
                nc.scalar.copy(ht[:, 512:520], psB[:])
                nc.sync.dma_start(out_dram[t * 128:(t + 1) * 128, :], ht[:])

        def zero_out(out_dram, width):
            for t in range(NLOC_PAD // 128):
                nc.sync.dma_start(out_dram[t * 128:(t + 1) * 128, :],
                                  zero640[:, 0:width])
            # remaining 64 rows
            nc.sync.dma_start(out_dram[SH:SH + 64, :], zero640[0:64, 0:width])

        def conv_edges(pool, psp, Htab, outd, maxk):
            """conv edge phase: gather/logits/segsum-matmul/scatter."""
            for ch in range(min(CONV_CHUNKS, max_ch)):
                TT = CHUNK_C // 128  # 16
                IW = CHUNK_C // 16   # idx cols per chunk
                G = pool.tile([128, TT, COLS], bf16, tag="G")
                ED = pool.tile([128, TT, 128], bf16, tag="ED")
                if no_gath:
                    nc.vector.memset(G[:], 0.25)
                    nc.vector.memset(ED[:], 0.25)
                else:
                    ns = CHUNK_C // 1024  # sub-gathers of 1024 idxs
                    for s_ in range(ns):
                        i0 = ch * IW + s_ * 64
                        t0 = s_ * 8
                        nc.gpsimd.dma_gather(
                            G[:, t0:t0 + 8, :], Htab[:, :],
                            gixc_s[:, i0:i0 + 64], 1024, 1024, COLS)
                        nc.gpsimd.dma_gather(
                            ED[:, t0:t0 + 8, :], Htab[:, 512:640],
                            gixd_s[:, i0:i0 + 64], 1024, 1024, 128,
                            elem_step=COLS)
                Ef = pool.tile([128, TT, HEADS], f32, tag="Ef")
                nc.vector.tensor_tensor(Ef[:], G[:, :, 512:516], ED[:, :, 4:8],
                                        mybir_.AluOpType.add)
                El = pool.tile([128, TT, HEADS], f32, tag="El")
                nc.vector.scalar_tensor_tensor(El[:], Ef[:], NEG, Ef[:],
                                               mybir_.AluOpType.mult,
                                               mybir_.AluOpType.max)
                Ab = pool.tile([128, TT, HEADS], bf16, tag="Ab")
                nc.scalar.activation(Ab[:], El[:],
                                     mybir_.ActivationFunctionType.Exp)
                nc.vector.memset(G[:, :, 512:516], 1.0)
                so = sofc_s[:, ch * TT:(ch + 1) * TT]
                I = pool.tile([128, TT, 128], bf16, tag="I")
                nc.vector.tensor_tensor(
                    I[:],
                    iota_f[:].unsqueeze(1).broadcast_to([128, TT, 128]),
                    so.unsqueeze(2).broadcast_to([128, TT, 128]),
                    mybir_.AluOpType.is_equal)
                for tt in range(TT):
                    t = ch * TT + tt
                    S = pool.tile([128, HEADS, 128], bf16, tag="S", bufs=4)
                    nc.vector.tensor_tensor(
                        S[:],
                        I[:, tt, :].unsqueeze(1).broadcast_to([128, HEADS, 128]),
                        Ab[:, tt, :].unsqueeze(2).broadcast_to([128, HEADS, 128]),
                        mybir_.AluOpType.mult)
                    scat = pool.tile([128, COLS], bf16, tag="scat", bufs=4)
                    for h in range(HEADS):
                        acc = psp.tile([128, 129], f32, space="PSUM", tag="accC", bufs=8)
                        nc.tensor.matmul(acc[:, 0:128], lhsT=S[:, h, :],
                                         rhs=G[:, tt, h * 128:(h + 1) * 128],
                                         start=True, stop=True)
                        nc.tensor.matmul(acc[:, 128:129], lhsT=S[:, h, :],
                                         rhs=G[:, tt, 512:513],
                                         start=True, stop=True)
                        if h % 2 == 0:
                            nc.vector.tensor_copy(
                                scat[:, h * 128:(h + 1) * 128], acc[:, 0:128])
                            nc.vector.tensor_copy(
                                scat[:, 512 + h:513 + h], acc[:, 128:129])
                        else:
                            nc.scalar.copy(
                                scat[:, h * 128:(h + 1) * 128], acc[:, 0:128])
                            nc.scalar.copy(
                                scat[:, 512 + h:513 + h], acc[:, 128:129])
                    if not no_scat:
                        nc.gpsimd.dma_scatter_add(
                            outd[:, :], scat[:].unsqueeze(1),
                            sixc_s[:, t * 8:(t + 1) * 8],
                            128, int(maxk[t]), COLS)
                    else:
                        nc.sync.dma_start(
                            outd[0:128, :], scat[:])

        def conv_div(pool, psp, outd, bsb, ysd, ypd, stash_T):
            """softmax divide + bias + relu; write row shard + dis-scaled shard;
            optionally stash transposed into y2T_sb."""
            for t in range(LT):
                ot = pool.tile([128, COLS], bf16, tag="ot")
                nc.sync.dma_start(ot[:], outd[t * 128:(t + 1) * 128, :])
                d4 = pool.tile([128, 4], f32, tag="d4")
                nc.vector.tensor_scalar_add(d4[:], ot[:, 512:516], EPS)
                dr = pool.tile([128, 4], f32, tag="dr")
                nc.vector.reciprocal(dr[:], d4[:])
                otf = pool.tile([128, 512], f32, tag="otf")
                nc.scalar.copy(otf[:], ot[:, 0:512])
                ym = pool.tile([128, HEADS, 128], f32, tag="ym")
                nc.vector.tensor_tensor(
                    ym[:],
                    otf[:].rearrange("p (h c) -> p h c", h=HEADS),
                    dr[:].unsqueeze(2).broadcast_to([128, HEADS, 128]),
                    mybir_.AluOpType.mult)
                yb = pool.tile([128, 512], f32, tag="yb")
                nc.vector.tensor_add(yb[:], ym[:].rearrange("p h c -> p (h c)"),
                                     bsb[:])
                yrow = pool.tile([128, 512], bf16, tag="yrow")
                nc.vector.tensor_scalar_max(yrow[:], yb[:], 0.0)
                nc.sync.dma_start(ysd[t * 128:(t + 1) * 128, :], yrow[:])
                ysc = pool.tile([128, 512], bf16, tag="ysc")
                nc.vector.tensor_scalar(ysc[:], yb[:], 0.0, dis_s[:, t:t + 1],
                                        mybir_.AluOpType.max,
                                        mybir_.AluOpType.mult)
                nc.sync.dma_start(ypd[t * 128:(t + 1) * 128, :], ysc[:])
                if stash_T:
                    for k in range(4):
                        pt = psp.tile([128, 128], bf16, space="PSUM", tag="ptT", bufs=2)
                        nc.tensor.transpose(pt[:], yrow[:, k * 128:(k + 1) * 128],
                                            ident[:])
                        nc.scalar.copy(y2T_sb[:, k, t * 128:(t + 1) * 128], pt[:])

        def lp_edges(pool, psp, Ytab, outd, maxk):
            for ch in range(LP_CHUNKS):
                TT = CHUNK_L // 128  # 32
                IW = CHUNK_L // 16
                G = pool.tile([128, TT, 512], bf16, tag="GL")
                for s_ in range(CHUNK_L // 1024):
                    i0 = ch * IW + s_ * 64
                    t0 = s_ * 8
                    nc.gpsimd.dma_gather(
                        G[:, t0:t0 + 8, :], Ytab[:, :],
                        gixl_s[:, i0:i0 + 64], 1024, 1024, 512)
                so = sofl_s[:, ch * TT:(ch + 1) * TT]
                I = pool.tile([128, TT, 128], bf16, tag="IL")
                nc.vector.tensor_tensor(
                    I[:],
                    iota_f[:].unsqueeze(1).broadcast_to([128, TT, 128]),
                    so.unsqueeze(2).broadcast_to([128, TT, 128]),
                    mybir_.AluOpType.is_equal)
                for tt in range(TT):
                    t = ch * TT + tt
                    acc = psp.tile([128, 512], f32, space="PSUM", tag="accL", bufs=4)
                    nc.tensor.matmul(acc[:], lhsT=I[:, tt, :], rhs=G[:, tt, :],
                                     start=True, stop=True)
                    scat = pool.tile([128, 512], bf16, tag="scatL", bufs=4)
                    if tt % 2 == 0:
                        nc.vector.tensor_copy(scat[:], acc[:])
                    else:
                        nc.scalar.copy(scat[:], acc[:])
                    nc.gpsimd.dma_scatter_add(
                        outd[:, :], scat[:].unsqueeze(1),
                        sixl_s[:, t * 8:(t + 1) * 8],
                        128, int(maxk[t]), 512)

        def lp_div(pool, psp, outd, resd, ypd, yrow_d, stash_T):
            """y = clip(dis*0.5*agg + 0.5*res, 0, 1); write scaled shard and
            optionally row shard / transposed stash."""
            for t in range(LT):
                ot = pool.tile([128, 512], bf16, tag="lot")
                nc.sync.dma_start(ot[:], outd[t * 128:(t + 1) * 128, :])
                rt = pool.tile([128, 512], bf16, tag="lrt")
                nc.sync.dma_start(rt[:], resd[t * 128:(t + 1) * 128, :])
                t1 = pool.tile([128, 512], f32, tag="lt1")
                nc.vector.tensor_scalar_mul(t1[:], ot[:], dish_s[:, t:t + 1])
                rtf = pool.tile([128, 512], f32, tag="lrtf")
                nc.scalar.mul(rtf[:], rt[:], 0.5)
                t2 = pool.tile([128, 512], f32, tag="lt2")
                nc.vector.tensor_add(t2[:], rtf[:], t1[:])
                yrow = pool.tile([128, 512], bf16, tag="lyrow")
                nc.vector.tensor_scalar(yrow[:], t2[:], 0.0, 1.0,
                                        mybir_.AluOpType.max,
                                        mybir_.AluOpType.min)
                if yrow_d is not None:
                    nc.sync.dma_start(yrow_d[t * 128:(t + 1) * 128, :], yrow[:])
                if ypd is not None:
                    ysc = pool.tile([128, 512], bf16, tag="lysc")
                    nc.vector.tensor_scalar_mul(ysc[:], yrow[:],
                                                dis_s[:, t:t + 1])
                    nc.sync.dma_start(ypd[t * 128:(t + 1) * 128, :], ysc[:])
                if stash_T:
                    for k in range(4):
                        pt = psp.tile([128, 128], bf16, space="PSUM", tag="ptT", bufs=2)
                        nc.tensor.transpose(pt[:], yrow[:, k * 128:(k + 1) * 128],
                                            ident[:])
                        nc.scalar.copy(y2T_sb[:, k, t * 128:(t + 1) * 128],
                                       pt[:])

        def allgather(shard_d, full_d):
            nc.gpsimd.collective_compute(AG, BYP, replica_groups=RG,
                                         ins=[shard_d[:, :]],
                                         outs=[full_d[:, :]])

        # ================= pipeline =================
        if phases >= 1:
            with tc.tile_pool(name="g1", bufs=2) as pool, \
                 tc.tile_pool(name="g1p", bufs=4, space="PSUM") as psp:
                def x_lhsT(pool, t, k):
                    lt = pool.tile([128, 128], bf16, tag="xlt", bufs=4)
                    nc.sync.dma_start(
                        lt[:], xTs[k * 128:(k + 1) * 128, t * 128:(t + 1) * 128])
                    return lt[:]
                gemm(pool, psp, x_lhsT, W1s, 2, H1ps)
        if phases >= 2:
            allgather(H1ps, H1p)

        if phases >= 3:
            with tc.tile_pool(name="c1", bufs=2) as pool, \
                 tc.tile_pool(name="c1p", bufs=8, space="PSUM") as psp:
                zero_out(OUTC1, COLS)
                conv_edges(pool, psp, H1p, OUTC1, conv_maxk)
                conv_div(pool, psp, OUTC1, b1s, y0s, shp[0], False)
        if phases >= 4:
            allgather(shp[0], Yt[0])

        if phases >= 5:
            with tc.tile_pool(name="l1", bufs=2) as pool, \
                 tc.tile_pool(name="l1p", bufs=4, space="PSUM") as psp:
                zero_out(OUTL[0], 512)
                lp_edges(pool, psp, Yt[0], OUTL[0], lp_maxk)
                lp_div(pool, psp, OUTL[0], y0s, shp[1], None, False)
        if phases >= 6:
            allgather(shp[1], Yt[1])
            with tc.tile_pool(name="l2", bufs=2) as pool, \
                 tc.tile_pool(name="l2p", bufs=4, space="PSUM") as psp:
                zero_out(OUTL[1], 512)
                lp_edges(pool, psp, Yt[1], OUTL[1], lp_maxk)
                lp_div(pool, psp, OUTL[1], y0s, None, y2s, True)

        if phases >= 7:
            with tc.tile_pool(name="g2", bufs=2) as pool, \
                 tc.tile_pool(name="g2p", bufs=4, space="PSUM") as psp:
                def y_lhsT(pool, t, k):
                    return y2T_sb[:, k, t * 128:(t + 1) * 128]
                gemm(pool, psp, y_lhsT, W2s, 4, H2ps)
            allgather(H2ps, H2p)

        if phases >= 8:
            with tc.tile_pool(name="c2", bufs=2) as pool, \
                 tc.tile_pool(name="c2p", bufs=8, space="PSUM") as psp:
                zero_out(OUTC2, COLS)
                conv_edges(pool, psp, H2p, OUTC2, conv_maxk)
                conv_div(pool, psp, OUTC2, b2s, z0s, shp[2], False)
            allgather(shp[2], Yt[2])

        if phases >= 9:
            with tc.tile_pool(name="l3", bufs=2) as pool, \
                 tc.tile_pool(name="l3p", bufs=4, space="PSUM") as psp:
                zero_out(OUTL[2], 512)
                lp_edges(pool, psp, Yt[2], OUTL[2], lp_maxk)
                lp_div(pool, psp, OUTL[2], z0s, shp[3], None, False)
            allgather(shp[3], Yt[3])
            with tc.tile_pool(name="l4", bufs=2) as pool, \
                 tc.tile_pool(name="l4p", bufs=4, space="PSUM") as psp:
                zero_out(OUTL[3], 512)
                lp_edges(pool, psp, Yt[3], OUTL[3], lp_maxk)
                lp_div(pool, psp, OUTL[3], z0s, None, z2s, False)

        if dbg:
            with tc.tile_pool(name="dbgp", bufs=2) as pool:
                pairs = [("dH1ps", H1ps, SH, COLS), ("dOUTC1", OUTC1, NLOC_PAD, COLS),
                         ("dy0s", y0s, SH, 512), ("dsh0", shp[0], SH, 512),
                         ("dOUTL0", OUTL[0], NLOC_PAD, 512), ("dsh1", shp[1], SH, 512),
                         ("dy2s", y2s, SH, 512), ("dH2ps", H2ps, SH, COLS),
                         ("dOUTC2", OUTC2, NLOC_PAD, COLS), ("dz0s", z0s, SH, 512),
                         ("dz2s", z2s, SH, 512)]
                for nm, ten, rr, cc in pairs:
                    if nm not in dbg_t:
                        continue
                    full = NLOC_PAD if rr == NLOC_PAD else SH
                    nt = full // 128
                    for t in range(nt):
                        ct = pool.tile([128, cc], bf16, tag="dbgt")
                        nc.sync.dma_start(ct[:], ten[t * 128:(t + 1) * 128, 0:cc])
                        nc.sync.dma_start(dbg_t[nm][t * 128:(t + 1) * 128, :], ct[:])
                    if rr == NLOC_PAD:
                        ct = pool.tile([128, cc], bf16, tag="dbgt")
                        nc.sync.dma_start(ct[0:64, :], ten[SH:SH + 64, 0:cc])
                        nc.sync.dma_start(dbg_t[nm][SH:SH + 64, :], ct[0:64, :])

        # ---- pooling ----
        with tc.tile_pool(name="pl", bufs=2) as pool, \
             tc.tile_pool(name="plp", bufs=2, space="PSUM") as psp:
            po = pool.tile([64, 1024], f32, tag="po")
            if phases >= 10:
                ps1 = psp.tile([64, 512], f32, space="PSUM", tag="ps1", bufs=1)
                ps2 = psp.tile([64, 512], f32, space="PSUM", tag="ps2", bufs=1)
                for t in range(LT):
                    IB = pool.tile([128, 64], bf16, tag="IB")
                    nc.vector.tensor_scalar(IB[:], iota64_f[:],
                                            batch_s[:, t:t + 1],
                                            None, mybir_.AluOpType.is_equal)
                    yt = pool.tile([128, 512], bf16, tag="pyt")
                    nc.sync.dma_start(yt[:], y2s[t * 128:(t + 1) * 128, :])
                    zt = pool.tile([128, 512], bf16, tag="pzt")
                    nc.sync.dma_start(zt[:], z2s[t * 128:(t + 1) * 128, :])
                    nc.tensor.matmul(ps1[:], lhsT=IB[:], rhs=yt[:],
                                     start=(t == 0), stop=(t == LT - 1))
                    nc.tensor.matmul(ps2[:], lhsT=IB[:], rhs=zt[:],
                                     start=(t == 0), stop=(t == LT - 1))
                nc.vector.tensor_copy(po[:, 0:512], ps1[:])
                nc.vector.tensor_copy(po[:, 512:1024], ps2[:])
            else:
                nc.vector.memset(po[:], 0.0)
            nc.sync.dma_start(pooled[:, :], po[:])

    nc.compile()
    return nc


# ---------------- entry ----------------

def kernel(x, edge_index, batch,
           conv1_W, conv1_asrc, conv1_adst, conv1_b,
           conv2_W, conv2_asrc, conv2_adst, conv2_b,
           mlp_W1, mlp_b1, mlp_W2, mlp_b2):
    from concourse.bass_utils import run_bass_kernel_spmd

    x = np.asarray(x, dtype=np.float32)
    batch_np = np.asarray(batch, dtype=np.int64)

    per_core, conv_maxk, lp_maxk, xp, dis = _preprocess(x, edge_index, batch)

    W1a = _fold_w(conv1_W, conv1_asrc, conv1_adst)
    W2a = _fold_w(conv2_W, conv2_asrc, conv2_adst)
    b1r = np.tile(np.asarray(conv1_b, np.float32)[None, :], (128, 1))
    b2r = np.tile(np.asarray(conv2_b, np.float32)[None, :], (128, 1))

    xT = np.ascontiguousarray(xp.T).astype(BF16)  # [256, NP]

    in_maps = []
    for c in range(N_CORES):
        pc = per_core[c]
        in_maps.append({
            "xTs": np.ascontiguousarray(xT[:, c * SH:(c + 1) * SH]),
            "W1": W1a, "W2": W2a, "bias1": b1r, "bias2": b2r,
            "gixc": pc["conv"]["gidx_w"], "gixd": pc["conv"]["didx_w"],
            "sixc": pc["conv"]["sidx_w"],
            "sofc": pc["conv"]["segoff_w"],
            "gixl": pc["lp"]["gidx_w"], "sixl": pc["lp"]["sidx_w"],
            "sofl": pc["lp"]["segoff_w"],
            "disd": pc["dis"], "dishd": pc["dish"], "batchd": pc["batch"],
        })

    import os
    phases = int(os.environ.get("PHASES", "10"))
    key = (f"nc{phases}-{os.environ.get('NO_SCAT')}-"
           f"{os.environ.get('NO_GATH')}-{os.environ.get('MAX_CH')}-"
           f"{os.environ.get('DBG')}")
    if key not in _cached:
        _cached[key] = _build(conv_maxk, lp_maxk, phases)
    nc = _cached[key]

    import time
    t0 = time.time()
    res = run_bass_kernel_spmd(nc, in_maps, core_ids=list(range(N_CORES)))
    _cached["device_wall_ns"] = int((time.time() - t0) * 1e9)
    _cached["last_result"] = res

    # ---- host postprocess ----
    pool_dev = np.zeros((NGR, 1024), dtype=np.float32)
    for c in range(N_CORES):
        pool_dev += np.asarray(res.results[c]["pooled"]).astype(np.float32)

    b = batch_np
    xsums = np.zeros((NGR, IN_CH), dtype=np.float32)
    np.add.at(xsums, b, x)
    cnts = np.bincount(b, minlength=NGR).astype(np.float32)
    cdiv = np.maximum(cnts, 1.0)[:, None]

    pooled_full = np.concatenate(
        [xsums, pool_dev[:, 0:512], pool_dev[:, 512:1024]], axis=1) / cdiv
    hdd = np.maximum(pooled_full @ np.asarray(mlp_W1, np.float32)
                     + np.asarray(mlp_b1, np.float32), 0.0)
    out = hdd @ np.asarray(mlp_W2, np.float32) + np.asarray(mlp_b2, np.float32)
    return out.astype(np.float32)


# revision 3
# speedup vs baseline: 6.4187x; 6.4187x over previous
"""DSGIAT GraphBranch: full-pipeline Bass kernel for 8 trn2 NeuronCores.

Node-sharded (3840 padded nodes/core).  Per core:
  GEMM (own shard) -> AllGather H table -> dst-sharded edge aggregation via
  dma_gather + indicator-matmul segment-sum + dma_scatter_add -> softmax
  division -> label-prop (separable sym-norm, indicator matmuls) with
  AllGather between steps -> pooling partials.  Host: preprocessing,
  x-pooling, final MLP.
"""
import numpy as np
import ml_dtypes
from contextlib import ExitStack

BF16 = ml_dtypes.bfloat16

# ---------------- problem constants ----------------
N = 30000
N_CORES = 8
SH_REAL = 3750           # real nodes per core
SH = 3840                # padded shard (30 * 128)
NP = SH * N_CORES        # 30720
LT = 30                  # local node tiles
IN_CH = 256
OUT1 = 512
HEADS = 4
HID = 128
COLS = 640               # H table width: 0:512 h | 512:516 es | 516:520 ed | pad
NGR = 64
EPS = 1e-16
NEG = 0.2
DUMP = SH                # scatter dump row
NLOC_PAD = SH + 64       # OUT tables rows (incl dump region)

CONV_CHUNKS = 32         # conv edge chunks of 2048 per core
LP_CHUNKS = 15
CHUNK_C = 2048
CHUNK_L = 4096
CONV_E = CONV_CHUNKS * CHUNK_C   # 65536 slots
LP_E = LP_CHUNKS * CHUNK_L       # 61440 slots
CONV_T = CONV_E // 128         # 512 tiles
LP_T = LP_E // 128             # 480 tiles

_cached = {}


# ---------------- host preprocessing ----------------

def _remap(n):
    """original node id -> padded id"""
    return (n // SH_REAL) * SH + (n % SH_REAL)


def _wrap_gidx(idx, nslots):
    """global gather indices -> [128, nslots//16] int16 (wrapped, replicated)."""
    a = np.zeros(nslots, dtype=np.int16)
    a[: len(idx)] = idx.astype(np.int16)
    w = a.reshape(nslots // 16, 16).T
    return np.ascontiguousarray(w)  # [16, n/16]; replicated on device


def _edge_meta(src_r, dst_r, nslots):
    """Per-core edge metadata for dst-sharded aggregation.

    src_r/dst_r: remapped global ids, globally sorted by dst_r.
    Returns per-core dicts + global maxk per tile.
    """
    ntiles = nslots // 128
    cores = []
    for c in range(N_CORES):
        lo, hi = c * SH, (c + 1) * SH
        m = (dst_r >= lo) & (dst_r < hi)
        s = src_r[m]
        d = dst_r[m] - lo
        ne = len(s)
        assert ne <= nslots, f"core {c}: {ne} > {nslots}"
        spad = np.zeros(nslots, dtype=np.int64)
        spad[:ne] = s
        dpad = np.full(nslots, 1 << 20, dtype=np.int64)  # pad sentinel
        dpad[:ne] = d
        pos = np.arange(nslots)
        newseg = np.ones(nslots, dtype=bool)
        newseg[1:] = (dpad[1:] != dpad[:-1])
        newseg |= (pos % 128 == 0)
        segg = np.cumsum(newseg) - 1
        tile_id = pos // 128
        tile_start_seg = segg[pos - (pos % 128)]
        segoff = segg - tile_start_seg          # [nslots]
        k = segoff[pos % 128 == 127] + 1        # per-tile seg count [ntiles]
        # sidx values: for each (tile, rank) the local dst (or DUMP for pad)
        sidx = np.full((ntiles, 128), -1, dtype=np.int64)
        first = newseg
        tv = tile_id[first]
        rv = segoff[first]
        dv = dpad[first]
        dv = np.where(dv == (1 << 20), DUMP, dv)
        sidx[tv, rv] = dv
        dglob = np.zeros(nslots, dtype=np.int64)
        dglob[:ne] = dst_r[m]
        cores.append(dict(s=spad, d=dglob, segoff=segoff.astype(np.float32),
                          sidx=sidx, k=k))
    maxk = np.max(np.stack([c["k"] for c in cores]), axis=0)  # [ntiles]
    for c in cores:
        sidx = c["sidx"]
        k = c["k"]
        for t in range(ntiles):
            if k[t] < maxk[t]:
                sidx[t, k[t]:maxk[t]] = DUMP
        # wrap: [ntiles, 128] -> [128, ntiles*8]
        w = sidx.astype(np.int16).reshape(ntiles, 8, 16).transpose(2, 0, 1)
        w = w.reshape(16, ntiles * 8)
        c["sidx_w"] = np.ascontiguousarray(w)
        # segoff wrapped: edge i of tile t -> partition i, col t
        c["segoff_w"] = np.ascontiguousarray(
            c["segoff"].reshape(ntiles, 128).T)
        c["gidx_w"] = _wrap_gidx(c["s"], nslots)
        c["didx_w"] = _wrap_gidx(c["d"], nslots)
    return cores, maxk


def _preprocess(x, edge_index, batch):
    src = np.asarray(edge_index[0], dtype=np.int64)
    dst = np.asarray(edge_index[1], dtype=np.int64)
    batch = np.asarray(batch, dtype=np.int64)

    deg = np.bincount(dst, minlength=N).astype(np.float32)
    dis = np.where(deg > 0, 1.0 / np.sqrt(np.maximum(deg, 1.0)), 0.0)

    src_r = _remap(src)
    dst_r = _remap(dst)
    loop = _remap(np.arange(N, dtype=np.int64))

    # conv edges (raw + self loops), sorted by dst
    cs = np.concatenate([src_r, loop])
    cd = np.concatenate([dst_r, loop])
    o = np.argsort(cd, kind="stable")
    conv_cores, conv_maxk = _edge_meta(cs[o], cd[o], CONV_E)

    # LP edges (raw), sorted by dst
    o2 = np.argsort(dst_r, kind="stable")
    lp_cores, lp_maxk = _edge_meta(src_r[o2], dst_r[o2], LP_E)

    # per-core node-local tables [128, 30]: local node l = t*128 + p
    dis_pad = np.zeros(NP, dtype=np.float32)
    dis_pad[_remap(np.arange(N))] = dis
    batch_pad = np.full(NP, -1.0, dtype=np.float32)
    batch_pad[_remap(np.arange(N))] = batch.astype(np.float32)

    per_core = []
    for c in range(N_CORES):
        dl = dis_pad[c * SH:(c + 1) * SH].reshape(LT, 128).T  # [128, 30]
        bl = batch_pad[c * SH:(c + 1) * SH].reshape(LT, 128).T
        per_core.append(dict(
            dis=np.ascontiguousarray(dl),
            dish=np.ascontiguousarray(dl * 0.5),
            batch=np.ascontiguousarray(bl),
            conv=conv_cores[c], lp=lp_cores[c]))

    xp = np.zeros((NP, IN_CH), dtype=np.float32)
    xp[_remap(np.arange(N))] = np.asarray(x, dtype=np.float32)
    return per_core, conv_maxk, lp_maxk, xp, dis


def _fold_w(W, a_src, a_dst):
    """[Fi, 512] + [4,128]x2 -> [Fi, 520] bf16 augmented weight."""
    W = np.asarray(W, np.float32)
    wes = np.stack([W[:, h * HID:(h + 1) * HID] @ np.asarray(a_src, np.float32)[h]
                    for h in range(HEADS)], axis=1)
    wed = np.stack([W[:, h * HID:(h + 1) * HID] @ np.asarray(a_dst, np.float32)[h]
                    for h in range(HEADS)], axis=1)
    return np.concatenate([W, wes, wed], axis=1).astype(BF16)


# ---------------- device program ----------------

def _build(conv_maxk, lp_maxk, phases=10):
    import os
    import concourse.tile as tile
    from concourse import bacc, mybir
    no_scat = os.environ.get("NO_SCAT", "0") == "1"
    no_gath = os.environ.get("NO_GATH", "0") == "1"
    max_ch = int(os.environ.get("MAX_CH", "9999"))

    f32 = mybir.dt.float32
    bf16 = mybir.dt.bfloat16
    i16 = mybir.dt.int16
    i32 = mybir.dt.int32
    AG = "AllGather"
    BYP = mybir.AluOpType.bypass
    RG = [list(range(N_CORES))]

    nc = bacc.Bacc("TRN2", target_bir_lowering=False, debug=False,
                   num_devices=N_CORES, dynamic_dma_scratch_size=32768)

    # ---- inputs ----
    xTs = nc.dram_tensor("xTs", [IN_CH, SH], bf16, kind="ExternalInput")
    W1 = nc.dram_tensor("W1", [IN_CH, 520], bf16, kind="ExternalInput")
    W2 = nc.dram_tensor("W2", [OUT1, 520], bf16, kind="ExternalInput")
    bias1 = nc.dram_tensor("bias1", [128, 512], f32, kind="ExternalInput")
    bias2 = nc.dram_tensor("bias2", [128, 512], f32, kind="ExternalInput")
    gixc = nc.dram_tensor("gixc", [16, CONV_E // 16], i16, kind="ExternalInput")
    gixd = nc.dram_tensor("gixd", [16, CONV_E // 16], i16, kind="ExternalInput")
    sixc = nc.dram_tensor("sixc", [16, CONV_T * 8], i16, kind="ExternalInput")
    sofc = nc.dram_tensor("sofc", [128, CONV_T], f32, kind="ExternalInput")
    gixl = nc.dram_tensor("gixl", [16, LP_E // 16], i16, kind="ExternalInput")
    sixl = nc.dram_tensor("sixl", [16, LP_T * 8], i16, kind="ExternalInput")
    sofl = nc.dram_tensor("sofl", [128, LP_T], f32, kind="ExternalInput")
    disd = nc.dram_tensor("disd", [128, LT], f32, kind="ExternalInput")
    dishd = nc.dram_tensor("dishd", [128, LT], f32, kind="ExternalInput")
    batchd = nc.dram_tensor("batchd", [128, LT], f32, kind="ExternalInput")

    # ---- internal DRAM ----
    H1ps = nc.dram_tensor("H1ps", [SH, COLS], bf16)
    H1p = nc.dram_tensor("H1p", [NP, COLS], bf16)
    H2ps = nc.dram_tensor("H2ps", [SH, COLS], bf16)
    H2p = nc.dram_tensor("H2p", [NP, COLS], bf16)
    OUTC1 = nc.dram_tensor("OUTC1", [NLOC_PAD, COLS], bf16)
    OUTC2 = nc.dram_tensor("OUTC2", [NLOC_PAD, COLS], bf16)
    y0s = nc.dram_tensor("y0s", [SH, 512], bf16)
    z0s = nc.dram_tensor("z0s", [SH, 512], bf16)
    y2s = nc.dram_tensor("y2s", [SH, 512], bf16)
    z2s = nc.dram_tensor("z2s", [SH, 512], bf16)
    shp = [nc.dram_tensor(f"sh{i}", [SH, 512], bf16) for i in range(4)]
    Yt = [nc.dram_tensor(f"Y{i}", [NP, 512], bf16) for i in range(4)]
    OUTL = [nc.dram_tensor(f"OUTL{i}", [NLOC_PAD, 512], bf16) for i in range(4)]
    pooled = nc.dram_tensor("pooled", [64, 1024], f32, kind="ExternalOutput")
    dbg = os.environ.get("DBG", "0") == "1"
    dbg_t = {}
    if dbg:
        for nm, (rr, cc) in dict(
                dH1ps=(SH, COLS), dOUTC1=(NLOC_PAD, COLS), dy0s=(SH, 512),
                dsh0=(SH, 512), dOUTL0=(NLOC_PAD, 512), dsh1=(SH, 512),
                dy2s=(SH, 512), dH2ps=(SH, COLS), dOUTC2=(NLOC_PAD, COLS),
                dz0s=(SH, 512), dz2s=(SH, 512)).items():
            dbg_t[nm] = nc.dram_tensor(nm, [rr, cc], bf16,
                                       kind="ExternalOutput")

    with tile.TileContext(nc) as tc, ExitStack() as ctx:
        st = ctx.enter_context(tc.tile_pool(name="st", bufs=1))
        mybir_ = mybir

        # ---------- static SBUF ----------
        iota_i = st.tile([128, 128], i32, tag="iota_i")
        nc.gpsimd.iota(iota_i[:], [[1, 128]], channel_multiplier=0)
        iota_f = st.tile([128, 128], f32, tag="iota_f")
        nc.vector.tensor_copy(iota_f[:], iota_i[:])
        iota_pm = st.tile([128, 128], i32, tag="iota_pm")
        nc.gpsimd.iota(iota_pm[:], [[1, 128]], channel_multiplier=-1)
        ident = st.tile([128, 128], bf16, tag="ident")
        nc.vector.tensor_single_scalar(ident[:], iota_pm[:], 0,
                                       mybir_.AluOpType.is_equal)
        iota64_f = st.tile([128, 64], f32, tag="iota64_f")
        nc.vector.tensor_copy(iota64_f[:], iota_i[:, 0:64])

        def load_const(t_dram, shape, dt, tg):
            t = st.tile(shape, dt, tag=tg, name=tg)
            nc.sync.dma_start(t[:], t_dram[:, :])
            return t

        def load_idx(t_dram, ncols, tg):
            t = st.tile([128, ncols], i16, tag=tg, name=tg)
            for k in range(8):
                nc.sync.dma_start(t[16 * k:16 * (k + 1), :], t_dram[:, :])
            return t

        W1s = st.tile([128, 2, 520], bf16, tag="W1s")
        nc.sync.dma_start(W1s[:, 0, :], W1[0:128, :])
        nc.sync.dma_start(W1s[:, 1, :], W1[128:256, :])
        W2s = st.tile([128, 4, 520], bf16, tag="W2s")
        for k in range(4):
            nc.sync.dma_start(W2s[:, k, :], W2[k * 128:(k + 1) * 128, :])
        b1s = load_const(bias1, [128, 512], f32, "b1s")
        b2s = load_const(bias2, [128, 512], f32, "b2s")
        gixc_s = load_idx(gixc, CONV_E // 16, "gixc_s")
        gixd_s = load_idx(gixd, CONV_E // 16, "gixd_s")
        sixc_s = load_idx(sixc, CONV_T * 8, "sixc_s")
        sofc_s = load_const(sofc, [128, CONV_T], f32, "sofc_s")
        gixl_s = load_idx(gixl, LP_E // 16, "gixl_s")
        sixl_s = load_idx(sixl, LP_T * 8, "sixl_s")
        sofl_s = load_const(sofl, [128, LP_T], f32, "sofl_s")
        dis_s = load_const(disd, [128, LT], f32, "dis_s")
        dish_s = load_const(dishd, [128, LT], f32, "dish_s")
        batch_s = load_const(batchd, [128, LT], f32, "batch_s")
        y2T_sb = st.tile([128, 4, SH], bf16, tag="y2T_sb")

        zero640 = st.tile([128, COLS], bf16, tag="zero640")
        nc.vector.memset(zero640[:], 0.0)

        # ---------- helpers ----------
        def gemm(pool, psp, src_lhsT, Wsb, nk, out_dram):
            """node-sharded GEMM: out[t*128+p, 0:520] = sum_k lhsT_k.T @ W."""
            for t in range(LT):
                psA = psp.tile([128, 512], f32, space="PSUM", tag="psA",
                               bufs=2)
                psB = psp.tile([128, 8], f32, space="PSUM", tag="psB", bufs=2)
                lts = [src_lhsT(pool, t, k) for k in range(nk)]
                for k in range(nk):
                    nc.tensor.matmul(psA[:], lhsT=lts[k], rhs=Wsb[:, k, 0:512],
                                     start=(k == 0), stop=(k == nk - 1))
                for k in range(nk):
                    nc.tensor.matmul(psB[:], lhsT=lts[k], rhs=Wsb[:, k, 512:520],
                                     start=(k == 0), stop=(k == nk - 1))
                ht = pool.tile([128, COLS], bf16, tag="gemm_out")
                nc.vector.tensor_copy(ht[:, 0:512], psA[:])
                nc.scalar.copy(ht[:, 512:520], psB[:])
                nc.sync.dma_start(out_dram[t * 128:(t + 1) * 128, :], ht[:])

        def zero_out(out_dram, width):
            for t in range(NLOC_PAD // 128):
                nc.sync.dma_start(out_dram[t * 128:(t + 1) * 128, :],
                                  zero640[:, 0:width])
            # remaining 64 rows
            nc.sync.dma_start(out_dram[SH:SH + 64, :], zero640[0:64, 0:width])

        def conv_edges(pool, psp, Htab, outd, maxk):
            """conv edge phase: gather/logits/segsum-matmul/scatter."""
            for ch in range(min(CONV_CHUNKS, max_ch)):
                TT = CHUNK_C // 128  # 16
                IW = CHUNK_C // 16   # idx cols per chunk
                G = pool.tile([128, TT, COLS], bf16, tag="G")
                ED = pool.tile([128, TT, 128], bf16, tag="ED")
                if no_gath:
                    nc.vector.memset(G[:], 0.25)
                    nc.vector.memset(ED[:], 0.25)
                else:
                    ns = CHUNK_C // 1024  # sub-gathers of 1024 idxs
                    for s_ in range(ns):
                        i0 = ch * IW + s_ * 64
                        t0 = s_ * 8
                        nc.gpsimd.dma_gather(
                            G[:, t0:t0 + 8, :], Htab[:, :],
                            gixc_s[:, i0:i0 + 64], 1024, 1024, COLS)
                        nc.gpsimd.dma_gather(
                            ED[:, t0:t0 + 8, :], Htab[:, 512:640],
                            gixd_s[:, i0:i0 + 64], 1024, 1024, 128,
                            elem_step=COLS)
                Ef = pool.tile([128, TT, HEADS], f32, tag="Ef")
                nc.vector.tensor_tensor(Ef[:], G[:, :, 512:516], ED[:, :, 4:8],
                                        mybir_.AluOpType.add)
                El = pool.tile([128, TT, HEADS], f32, tag="El")
                nc.vector.scalar_tensor_tensor(El[:], Ef[:], NEG, Ef[:],
                                               mybir_.AluOpType.mult,
                                               mybir_.AluOpType.max)
                Ab = pool.tile([128, TT, HEADS], bf16, tag="Ab")
                nc.scalar.activation(Ab[:], El[:],
                                     mybir_.ActivationFunctionType.Exp)
                nc.vector.memset(G[:, :, 512:516], 1.0)
                so = sofc_s[:, ch * TT:(ch + 1) * TT]
                I = pool.tile([128, TT, 128], bf16, tag="I")
                nc.vector.tensor_tensor(
                    I[:],
                    iota_f[:].unsqueeze(1).broadcast_to([128, TT, 128]),
                    so.unsqueeze(2).broadcast_to([128, TT, 128]),
                    mybir_.AluOpType.is_equal)
                for tt in range(TT):
                    t = ch * TT + tt
                    S = pool.tile([128, HEADS, 128], bf16, tag="S", bufs=4)
                    nc.vector.tensor_tensor(
                        S[:],
                        I[:, tt, :].unsqueeze(1).broadcast_to([128, HEADS, 128]),
                        Ab[:, tt, :].unsqueeze(2).broadcast_to([128, HEADS, 128]),
                        mybir_.AluOpType.mult)
                    scat = pool.tile([128, COLS], bf16, tag="scat", bufs=4)
                    for h in range(HEADS):
                        acc = psp.tile([128, 129], f32, space="PSUM", tag="accC", bufs=8)
                        nc.tensor.matmul(acc[:, 0:128], lhsT=S[:, h, :],
                                         rhs=G[:, tt, h * 128:(h + 1) * 128],
                                         start=True, stop=True)
                        nc.tensor.matmul(acc[:, 128:129], lhsT=S[:, h, :],
                                         rhs=G[:, tt, 512:513],
                                         start=True, stop=True)
                        if h % 2 == 0:
                            nc.vector.tensor_copy(
                                scat[:, h * 128:(h + 1) * 128], acc[:, 0:128])
                            nc.vector.tensor_copy(
                                scat[:, 512 + h:513 + h], acc[:, 128:129])
                        else:
                            nc.scalar.copy(
                                scat[:, h * 128:(h + 1) * 128], acc[:, 0:128])
                            nc.scalar.copy(
                                scat[:, 512 + h:513 + h], acc[:, 128:129])
                    if not no_scat:
                        nc.gpsimd.dma_scatter_add(
                            outd[:, :], scat[:].unsqueeze(1),
                            sixc_s[:, t * 8:(t + 1) * 8],
                            128, int(maxk[t]), COLS)
                    else:
                        nc.sync.dma_start(
                            outd[0:128, :], scat[:])

        def conv_div(pool, psp, outd, bsb, ysd, ypd, stash_T):
            """softmax divide + bias + relu; write row shard + dis-scaled shard;
            optionally stash transposed into y2T_sb."""
            for t in range(LT):
                ot = pool.tile([128, COLS], bf16, tag="ot")
                nc.sync.dma_start(ot[:], outd[t * 128:(t + 1) * 128, :])
                d4 = pool.tile([128, 4], f32, tag="d4")
                nc.vector.tensor_scalar_add(d4[:], ot[:, 512:516], EPS)
                dr = pool.tile([128, 4], f32, tag="dr")
                nc.vector.reciprocal(dr[:], d4[:])
                otf = pool.tile([128, 512], f32, tag="otf")
                nc.scalar.copy(otf[:], ot[:, 0:512])
                ym = pool.tile([128, HEADS, 128], f32, tag="ym")
                nc.vector.tensor_tensor(
                    ym[:],
                    otf[:].rearrange("p (h c) -> p h c", h=HEADS),
                    dr[:].unsqueeze(2).broadcast_to([128, HEADS, 128]),
                    mybir_.AluOpType.mult)
                yb = pool.tile([128, 512], f32, tag="yb")
                nc.vector.tensor_add(yb[:], ym[:].rearrange("p h c -> p (h c)"),
                                     bsb[:])
                yrow = pool.tile([128, 512], bf16, tag="yrow")
                nc.vector.tensor_scalar_max(yrow[:], yb[:], 0.0)
                nc.sync.dma_start(ysd[t * 128:(t + 1) * 128, :], yrow[:])
                ysc = pool.tile([128, 512], bf16, tag="ysc")
                nc.vector.tensor_scalar(ysc[:], yb[:], 0.0, dis_s[:, t:t + 1],
                                        mybir_.AluOpType.max,
                                        mybir_.AluOpType.mult)
                nc.sync.dma_start(ypd[t * 128:(t + 1) * 128, :], ysc[:])
                if stash_T:
                    for k in range(4):
                        pt = psp.tile([128, 128], bf16, space="PSUM", tag="ptT", bufs=2)
                        nc.tensor.transpose(pt[:], yrow[:, k * 128:(k + 1) * 128],
                                            ident[:])
                        nc.scalar.copy(y2T_sb[:, k, t * 128:(t + 1) * 128], pt[:])

        def lp_edges(pool, psp, Ytab, outd, maxk):
            for ch in range(LP_CHUNKS):
                TT = CHUNK_L // 128  # 32
                IW = CHUNK_L // 16
                G = pool.tile([128, TT, 512], bf16, tag="GL")
                for s_ in range(CHUNK_L // 1024):
                    i0 = ch * IW + s_ * 64
                    t0 = s_ * 8
                    nc.gpsimd.dma_gather(
                        G[:, t0:t0 + 8, :], Ytab[:, :],
                        gixl_s[:, i0:i0 + 64], 1024, 1024, 512)
                so = sofl_s[:, ch * TT:(ch + 1) * TT]
                I = pool.tile([128, TT, 128], bf16, tag="IL")
                nc.vector.tensor_tensor(
                    I[:],
                    iota_f[:].unsqueeze(1).broadcast_to([128, TT, 128]),
                    so.unsqueeze(2).broadcast_to([128, TT, 128]),
                    mybir_.AluOpType.is_equal)
                for tt in range(TT):
                    t = ch * TT + tt
                    acc = psp.tile([128, 512], f32, space="PSUM", tag="accL", bufs=4)
                    nc.tensor.matmul(acc[:], lhsT=I[:, tt, :], rhs=G[:, tt, :],
                                     start=True, stop=True)
                    scat = pool.tile([128, 512], bf16, tag="scatL", bufs=4)
                    if tt % 2 == 0:
                        nc.vector.tensor_copy(scat[:], acc[:])
                    else:
                        nc.scalar.copy(scat[:], acc[:])
                    nc.gpsimd.dma_scatter_add(
                        outd[:, :], scat[:].unsqueeze(1),
                        sixl_s[:, t * 8:(t + 1) * 8],
                        128, int(maxk[t]), 512)

        def lp_div(pool, psp, outd, resd, ypd, yrow_d, stash_T):
            """y = clip(dis*0.5*agg + 0.5*res, 0, 1); write scaled shard and
            optionally row shard / transposed stash."""
            for t in range(LT):
                ot = pool.tile([128, 512], bf16, tag="lot")
                nc.sync.dma_start(ot[:], outd[t * 128:(t + 1) * 128, :])
                rt = pool.tile([128, 512], bf16, tag="lrt")
                nc.sync.dma_start(rt[:], resd[t * 128:(t + 1) * 128, :])
                t1 = pool.tile([128, 512], f32, tag="lt1")
                nc.vector.tensor_scalar_mul(t1[:], ot[:], dish_s[:, t:t + 1])
                rtf = pool.tile([128, 512], f32, tag="lrtf")
                nc.scalar.mul(rtf[:], rt[:], 0.5)
                t2 = pool.tile([128, 512], f32, tag="lt2")
                nc.vector.tensor_add(t2[:], rtf[:], t1[:])
                yrow = pool.tile([128, 512], bf16, tag="lyrow")
                nc.vector.tensor_scalar(yrow[:], t2[:], 0.0, 1.0,
                                        mybir_.AluOpType.max,
                                        mybir_.AluOpType.min)
                if yrow_d is not None:
                    nc.sync.dma_start(yrow_d[t * 128:(t + 1) * 128, :], yrow[:])
                if ypd is not None:
                    ysc = pool.tile([128, 512], bf16, tag="lysc")
                    nc.vector.tensor_scalar_mul(ysc[:], yrow[:],
                                                dis_s[:, t:t + 1])
                    nc.sync.dma_start(ypd[t * 128:(t + 1) * 128, :], ysc[:])
                if stash_T:
                    for k in range(4):
                        pt = psp.tile([128, 128], bf16, space="PSUM", tag="ptT", bufs=2)
                        nc.tensor.transpose(pt[:], yrow[:, k * 128:(k + 1) * 128],
                                            ident[:])
                        nc.scalar.copy(y2T_sb[:, k, t * 128:(t + 1) * 128],
                                       pt[:])

        def allgather(shard_d, full_d):
            nc.gpsimd.collective_compute(AG, BYP, replica_groups=RG,
                                         ins=[shard_d[:, :]],
                                         outs=[full_d[:, :]])

        # ================= pipeline =================
        if phases >= 1:
            with tc.tile_pool(name="g1", bufs=2) as pool, \
                 tc.tile_pool(name="g1p", bufs=4, space="PSUM") as psp:
                def x_lhsT(pool, t, k):
                    lt = pool.tile([128, 128], bf16, tag="xlt", bufs=4)
                    nc.sync.dma_start(
                        lt[:], xTs[k * 128:(k + 1) * 128, t * 128:(t + 1) * 128])
                    return lt[:]
                gemm(pool, psp, x_lhsT, W1s, 2, H1ps)
        if phases >= 2:
            allgather(H1ps, H1p)

        if phases >= 3:
            with tc.tile_pool(name="c1", bufs=2) as pool, \
                 tc.tile_pool(name="c1p", bufs=8, space="PSUM") as psp:
                zero_out(OUTC1, COLS)
                conv_edges(pool, psp, H1p, OUTC1, conv_maxk)
                conv_div(pool, psp, OUTC1, b1s, y0s, shp[0], False)
        if phases >= 4:
            allgather(shp[0], Yt[0])

        if phases >= 5:
            with tc.tile_pool(name="l1", bufs=2) as pool, \
                 tc.tile_pool(name="l1p", bufs=4, space="PSUM") as psp:
                zero_out(OUTL[0], 512)
                lp_edges(pool, psp, Yt[0], OUTL[0], lp_maxk)
                lp_div(pool, psp, OUTL[0], y0s, shp[1], None, False)
        if phases >= 6:
            allgather(shp[1], Yt[1])
            with tc.tile_pool(name="l2", bufs=2) as pool, \
                 tc.tile_pool(name="l2p", bufs=4, space="PSUM") as psp:
                zero_out(OUTL[1], 512)
                lp_edges(pool, psp, Yt[1], OUTL[1], lp_maxk)
                lp_div(pool, psp, OUTL[1], y0s, None, y2s, True)

        if phases >= 7:
            with tc.tile_pool(name="g2", bufs=2) as pool, \
                 tc.tile_pool(name="g2p", bufs=4, space="PSUM") as psp:
                def y_lhsT(pool, t, k):
                    return y2T_sb[:, k, t * 128:(t + 1) * 128]
                gemm(pool, psp, y_lhsT, W2s, 4, H2ps)
            allgather(H2ps, H2p)

        if phases >= 8:
            with tc.tile_pool(name="c2", bufs=2) as pool, \
                 tc.tile_pool(name="c2p", bufs=8, space="PSUM") as psp:
                zero_out(OUTC2, COLS)
                conv_edges(pool, psp, H2p, OUTC2, conv_maxk)
                conv_div(pool, psp, OUTC2, b2s, z0s, shp[2], False)
            allgather(shp[2], Yt[2])

        if phases >= 9:
            with tc.tile_pool(name="l3", bufs=2) as pool, \
                 tc.tile_pool(name="l3p", bufs=4, space="PSUM") as psp:
                zero_out(OUTL[2], 512)
                lp_edges(pool, psp, Yt[2], OUTL[2], lp_maxk)
                lp_div(pool, psp, OUTL[2], z0s, shp[3], None, False)
            allgather(shp[3], Yt[3])
            with tc.tile_pool(name="l4", bufs=2) as pool, \
                 tc.tile_pool(name="l4p", bufs=4, space="PSUM") as psp:
                zero_out(OUTL[3], 512)
                lp_edges(pool, psp, Yt[3], OUTL[3], lp_maxk)
                lp_div(pool, psp, OUTL[3], z0s, None, z2s, False)

        if dbg:
            with tc.tile_pool(name="dbgp", bufs=2) as pool:
                pairs = [("dH1ps", H1ps, SH, COLS), ("dOUTC1", OUTC1, NLOC_PAD, COLS),
                         ("dy0s", y0s, SH, 512), ("dsh0", shp[0], SH, 512),
                         ("dOUTL0", OUTL[0], NLOC_PAD, 512), ("dsh1", shp[1], SH, 512),
                         ("dy2s", y2s, SH, 512), ("dH2ps", H2ps, SH, COLS),
                         ("dOUTC2", OUTC2, NLOC_PAD, COLS), ("dz0s", z0s, SH, 512),
                         ("dz2s", z2s, SH, 512)]
                for nm, ten, rr, cc in pairs:
                    if nm not in dbg_t:
                        continue
                    full = NLOC_PAD if rr == NLOC_PAD else SH
                    nt = full // 128
                    for t in range(nt):
                        ct = pool.tile([128, cc], bf16, tag="dbgt")
                        nc.sync.dma_start(ct[:], ten[t * 128:(t + 1) * 128, 0:cc])
                        nc.sync.dma_start(dbg_t[nm][t * 128:(t + 1) * 128, :], ct[:])
                    if rr == NLOC_PAD:
                        ct = pool.tile([128, cc], bf16, tag="dbgt")
                        nc.sync.dma_start(ct[0:64, :], ten[SH:SH + 64, 0:cc])
                        nc.sync.dma_start(dbg_t[nm][SH:SH + 64, :], ct[0:64, :])

        # ---- pooling ----
        with tc.tile_pool(name="pl", bufs=2) as pool, \
             tc.tile_pool(name="plp", bufs=2, space="PSUM") as psp:
            po = pool.tile([64, 1024], f32, tag="po")
            if phases >= 10:
                ps1 = psp.tile([64, 512], f32, space="PSUM", tag="ps1", bufs=1)
                ps2 = psp.tile([64, 512], f32, space="PSUM", tag="ps2", bufs=1)
                for t in range(LT):
                    IB = pool.tile([128, 64], bf16, tag="IB")
                    nc.vector.tensor_scalar(IB[:], iota64_f[:],
                                            batch_s[:, t:t + 1],
                                            None, mybir_.AluOpType.is_equal)
                    yt = pool.tile([128, 512], bf16, tag="pyt")
                    nc.sync.dma_start(yt[:], y2s[t * 128:(t + 1) * 128, :])
                    zt = pool.tile([128, 512], bf16, tag="pzt")
                    nc.sync.dma_start(zt[:], z2s[t * 128:(t + 1) * 128, :])
                    nc.tensor.matmul(ps1[:], lhsT=IB[:], rhs=yt[:],
                                     start=(t == 0), stop=(t == LT - 1))
                    nc.tensor.matmul(ps2[:], lhsT=IB[:], rhs=zt[:],
                                     start=(t == 0), stop=(t == LT - 1))
                nc.vector.tensor_copy(po[:, 0:512], ps1[:])
                nc.vector.tensor_copy(po[:, 512:1024], ps2[:])
            else:
                nc.vector.memset(po[:], 0.0)
            nc.sync.dma_start(pooled[:, :], po[:])

    nc.compile()
    return nc


# ---------------- entry ----------------

def _make_runner(nc):
    """Cached jitted shard_map runner (run_bass_via_pjrt rebuilds its jit
    every call -> re-trace + XLA compile each time; we build once)."""
    import jax
    import numpy as _np
    from jax.experimental.shard_map import shard_map
    from jax.sharding import Mesh, PartitionSpec
    from concourse import mybir
    from concourse.bass2jax import (_bass_exec_p, partition_id_tensor,
                                    install_neuronx_cc_hook)
    install_neuronx_cc_hook()

    partition_name = (nc.partition_id_tensor.name
                      if nc.partition_id_tensor else None)
    in_names, out_names, out_avals, zero_shapes = [], [], [], []
    for alloc in nc.m.functions[0].allocations:
        if not isinstance(alloc, mybir.MemoryLocationSet):
            continue
        name = alloc.memorylocations[0].name
        if alloc.kind == "ExternalInput":
            if name != partition_name:
                in_names.append(name)
        elif alloc.kind == "ExternalOutput":
            out_names.append(name)
            shape = tuple(alloc.tensor_shape)
            dtype = mybir.dt.np(alloc.dtype)
            out_avals.append(jax.core.ShapedArray(shape, dtype))
            zero_shapes.append((shape, dtype))
    n_params = len(in_names)
    n_outs = len(out_avals)
    all_in = list(in_names) + list(out_names)
    if partition_name is not None:
        all_in.append(partition_name)
    donate = tuple(range(n_params, n_params + n_outs))

    def _body(*args):
        operands = list(args)
        if partition_name is not None:
            operands.append(partition_id_tensor())
        outs = _bass_exec_p.bind(
            *operands,
            out_avals=tuple(out_avals),
            in_names=tuple(all_in),
            out_names=tuple(out_names),
            lowering_input_output_aliases=(),
            sim_require_finite=True,
            sim_require_nnan=True,
            nc=nc,
        )
        return tuple(outs)

    devices = jax.devices()[:N_CORES]
    mesh = Mesh(_np.asarray(devices), ("core",))
    in_specs = (PartitionSpec("core"),) * (n_params + n_outs)
    out_specs = (PartitionSpec("core"),) * n_outs
    sharded = jax.jit(
        shard_map(_body, mesh=mesh, in_specs=in_specs, out_specs=out_specs,
                  check_rep=False),
        donate_argnums=donate, keep_unused=True)

    def run(in_maps):
        per_core = [[_np.asarray(m[name]) for name in in_names]
                    for m in in_maps]
        concat_in = [
            _np.concatenate([per_core[c][i] for c in range(N_CORES)], axis=0)
            for i in range(n_params)]
        concat_zeros = [
            _np.zeros((N_CORES * s[0], *s[1:]), d) for (s, d) in zero_shapes]
        out_arrs = sharded(*concat_in, *concat_zeros)
        return [
            {name: _np.asarray(out_arrs[i]).reshape(
                N_CORES, *out_avals[i].shape)[c]
             for i, name in enumerate(out_names)}
            for c in range(N_CORES)]

    return run


class _Res:
    def __init__(self, results):
        self.results = results
        self.exec_time_ns = None


def kernel(x, edge_index, batch,
           conv1_W, conv1_asrc, conv1_adst, conv1_b,
           conv2_W, conv2_asrc, conv2_adst, conv2_b,
           mlp_W1, mlp_b1, mlp_W2, mlp_b2):

    x = np.asarray(x, dtype=np.float32)
    batch_np = np.asarray(batch, dtype=np.int64)

    per_core, conv_maxk, lp_maxk, xp, dis = _preprocess(x, edge_index, batch)

    W1a = _fold_w(conv1_W, conv1_asrc, conv1_adst)
    W2a = _fold_w(conv2_W, conv2_asrc, conv2_adst)
    b1r = np.tile(np.asarray(conv1_b, np.float32)[None, :], (128, 1))
    b2r = np.tile(np.asarray(conv2_b, np.float32)[None, :], (128, 1))

    xT = np.ascontiguousarray(xp.T).astype(BF16)  # [256, NP]

    in_maps = []
    for c in range(N_CORES):
        pc = per_core[c]
        in_maps.append({
            "xTs": np.ascontiguousarray(xT[:, c * SH:(c + 1) * SH]),
            "W1": W1a, "W2": W2a, "bias1": b1r, "bias2": b2r,
            "gixc": pc["conv"]["gidx_w"], "gixd": pc["conv"]["didx_w"],
            "sixc": pc["conv"]["sidx_w"],
            "sofc": pc["conv"]["segoff_w"],
            "gixl": pc["lp"]["gidx_w"], "sixl": pc["lp"]["sidx_w"],
            "sofl": pc["lp"]["segoff_w"],
            "disd": pc["dis"], "dishd": pc["dish"], "batchd": pc["batch"],
        })

    import os
    phases = int(os.environ.get("PHASES", "10"))
    key = (f"nc{phases}-{os.environ.get('NO_SCAT')}-"
           f"{os.environ.get('NO_GATH')}-{os.environ.get('MAX_CH')}-"
           f"{os.environ.get('DBG')}")
    if key not in _cached:
        _cached[key] = _build(conv_maxk, lp_maxk, phases)
    nc = _cached[key]

    rkey = key + "-runner"
    if rkey not in _cached:
        _cached[rkey] = _make_runner(nc)
    import time
    t0 = time.time()
    res = _Res(_cached[rkey](in_maps))
    _cached["device_wall_ns"] = int((time.time() - t0) * 1e9)
    _cached["last_result"] = res
    if os.environ.get("KTIME"):
        print(f"[ktime] device run: {_cached['device_wall_ns'] / 1e6:.1f} ms")

    # ---- host postprocess ----
    pool_dev = np.zeros((NGR, 1024), dtype=np.float32)
    for c in range(N_CORES):
        pool_dev += np.asarray(res.results[c]["pooled"]).astype(np.float32)

    b = batch_np
    xsums = np.zeros((NGR, IN_CH), dtype=np.float32)
    np.add.at(xsums, b, x)
    cnts = np.bincount(b, minlength=NGR).astype(np.float32)
    cdiv = np.maximum(cnts, 1.0)[:, None]

    pooled_full = np.concatenate(
        [xsums, pool_dev[:, 0:512], pool_dev[:, 512:1024]], axis=1) / cdiv
    hdd = np.maximum(pooled_full @ np.asarray(mlp_W1, np.float32)
                     + np.asarray(mlp_b1, np.float32), 0.0)
    out = hdd @ np.asarray(mlp_W2, np.float32) + np.asarray(mlp_b2, np.float32)
    return out.astype(np.float32)


# revision 4
# speedup vs baseline: 8.5260x; 1.3283x over previous
"""DSGIAT GraphBranch: full-pipeline Bass kernel for 8 trn2 NeuronCores.

Node-sharded (3840 padded nodes/core).  Per core:
  GEMM (own shard) -> AllGather H table -> dst-sharded edge aggregation via
  dma_gather + indicator-matmul segment-sum + dma_scatter_add -> softmax
  division -> label-prop (separable sym-norm, indicator matmuls) with
  AllGather between steps -> pooling partials.  Host: preprocessing,
  x-pooling, final MLP.
"""
import numpy as np
import ml_dtypes
from contextlib import ExitStack

BF16 = ml_dtypes.bfloat16

# ---------------- problem constants ----------------
N = 30000
N_CORES = 8
SH_REAL = 3750           # real nodes per core
SH = 3840                # padded shard (30 * 128)
NP = SH * N_CORES        # 30720
LT = 30                  # local node tiles
IN_CH = 256
OUT1 = 512
HEADS = 4
HID = 128
COLS = 640               # H table width: 0:512 h | 512:516 es | 516:520 ed | pad
NGR = 64
EPS = 1e-16
NEG = 0.2
DUMP = SH                # scatter dump row
NLOC_PAD = SH + 64       # OUT tables rows (incl dump region)

CONV_CHUNKS = 32         # conv edge chunks of 2048 per core
LP_CHUNKS = 15
CHUNK_C = 2048
CHUNK_L = 4096
CONV_E = CONV_CHUNKS * CHUNK_C   # 65536 slots
LP_E = LP_CHUNKS * CHUNK_L       # 61440 slots
CONV_T = CONV_E // 128         # 512 tiles
LP_T = LP_E // 128             # 480 tiles

_cached = {}


# ---------------- host preprocessing ----------------

def _remap(n):
    """original node id -> padded id"""
    return (n // SH_REAL) * SH + (n % SH_REAL)


def _wrap_gidx(idx, nslots):
    """global gather indices -> [128, nslots//16] int16 (wrapped, replicated)."""
    a = np.zeros(nslots, dtype=np.int16)
    a[: len(idx)] = idx.astype(np.int16)
    w = a.reshape(nslots // 16, 16).T
    return np.ascontiguousarray(w)  # [16, n/16]; replicated on device


def _edge_meta(src_r, dst_r, nslots):
    """Per-core edge metadata for dst-sharded aggregation.

    src_r/dst_r: remapped global ids, globally sorted by dst_r.
    Returns per-core dicts + global maxk per tile.
    """
    ntiles = nslots // 128
    cores = []
    for c in range(N_CORES):
        lo, hi = c * SH, (c + 1) * SH
        m = (dst_r >= lo) & (dst_r < hi)
        s = src_r[m]
        d = dst_r[m] - lo
        ne = len(s)
        assert ne <= nslots, f"core {c}: {ne} > {nslots}"
        spad = np.zeros(nslots, dtype=np.int64)
        spad[:ne] = s
        dpad = np.full(nslots, 1 << 20, dtype=np.int64)  # pad sentinel
        dpad[:ne] = d
        pos = np.arange(nslots)
        newseg = np.ones(nslots, dtype=bool)
        newseg[1:] = (dpad[1:] != dpad[:-1])
        newseg |= (pos % 128 == 0)
        segg = np.cumsum(newseg) - 1
        tile_id = pos // 128
        tile_start_seg = segg[pos - (pos % 128)]
        segoff = segg - tile_start_seg          # [nslots]
        k = segoff[pos % 128 == 127] + 1        # per-tile seg count [ntiles]
        # sidx values: for each (tile, rank) the local dst (or DUMP for pad)
        sidx = np.full((ntiles, 128), -1, dtype=np.int64)
        first = newseg
        tv = tile_id[first]
        rv = segoff[first]
        dv = dpad[first]
        dv = np.where(dv == (1 << 20), DUMP, dv)
        sidx[tv, rv] = dv
        dglob = np.zeros(nslots, dtype=np.int64)
        dglob[:ne] = dst_r[m]
        cores.append(dict(s=spad, d=dglob, segoff=segoff.astype(np.float32),
                          sidx=sidx, k=k))
    maxk = np.max(np.stack([c["k"] for c in cores]), axis=0)  # [ntiles]
    cols = np.arange(128)[None, :]
    for c in cores:
        sidx = c["sidx"]
        k = c["k"]
        fill = (cols >= k[:, None]) & (cols < maxk[:, None])
        sidx[fill] = DUMP
        # wrap: [ntiles, 128] -> [128, ntiles*8]
        w = sidx.astype(np.int16).reshape(ntiles, 8, 16).transpose(2, 0, 1)
        w = w.reshape(16, ntiles * 8)
        c["sidx_w"] = np.ascontiguousarray(w)
        # segoff wrapped: edge i of tile t -> partition i, col t
        c["segoff_w"] = np.ascontiguousarray(
            c["segoff"].reshape(ntiles, 128).T)
        c["gidx_w"] = _wrap_gidx(c["s"], nslots)
        c["didx_w"] = _wrap_gidx(c["d"], nslots)
    return cores, maxk


def _preprocess(x, edge_index, batch):
    src = np.asarray(edge_index[0], dtype=np.int64)
    dst = np.asarray(edge_index[1], dtype=np.int64)
    batch = np.asarray(batch, dtype=np.int64)

    deg = np.bincount(dst, minlength=N).astype(np.float32)
    dis = np.where(deg > 0, 1.0 / np.sqrt(np.maximum(deg, 1.0)), 0.0)

    src_r = _remap(src)
    dst_r = _remap(dst)
    loop = _remap(np.arange(N, dtype=np.int64))

    # conv edges (raw + self loops), sorted by dst
    cs = np.concatenate([src_r, loop])
    cd = np.concatenate([dst_r, loop])
    o = np.argsort(cd, kind="stable")
    conv_cores, conv_maxk = _edge_meta(cs[o], cd[o], CONV_E)

    # LP edges (raw), sorted by dst
    o2 = np.argsort(dst_r, kind="stable")
    lp_cores, lp_maxk = _edge_meta(src_r[o2], dst_r[o2], LP_E)

    # per-core node-local tables [128, 30]: local node l = t*128 + p
    dis_pad = np.zeros(NP, dtype=np.float32)
    dis_pad[_remap(np.arange(N))] = dis
    batch_pad = np.full(NP, -1.0, dtype=np.float32)
    batch_pad[_remap(np.arange(N))] = batch.astype(np.float32)

    per_core = []
    for c in range(N_CORES):
        dl = dis_pad[c * SH:(c + 1) * SH].reshape(LT, 128).T  # [128, 30]
        bl = batch_pad[c * SH:(c + 1) * SH].reshape(LT, 128).T
        per_core.append(dict(
            dis=np.ascontiguousarray(dl),
            dish=np.ascontiguousarray(dl * 0.5),
            batch=np.ascontiguousarray(bl),
            conv=conv_cores[c], lp=lp_cores[c]))

    xp = np.zeros((NP, IN_CH), dtype=np.float32)
    xp[_remap(np.arange(N))] = np.asarray(x, dtype=np.float32)
    return per_core, conv_maxk, lp_maxk, xp, dis


def _fold_w(W, a_src, a_dst):
    """[Fi, 512] + [4,128]x2 -> [Fi, 520] bf16 augmented weight."""
    W = np.asarray(W, np.float32)
    wes = np.stack([W[:, h * HID:(h + 1) * HID] @ np.asarray(a_src, np.float32)[h]
                    for h in range(HEADS)], axis=1)
    wed = np.stack([W[:, h * HID:(h + 1) * HID] @ np.asarray(a_dst, np.float32)[h]
                    for h in range(HEADS)], axis=1)
    return np.concatenate([W, wes, wed], axis=1).astype(BF16)


# ---------------- device program ----------------

def _build(conv_maxk, lp_maxk, phases=10):
    import os
    import concourse.tile as tile
    from concourse import bacc, mybir
    no_scat = os.environ.get("NO_SCAT", "0") == "1"
    no_gath = os.environ.get("NO_GATH", "0") == "1"
    max_ch = int(os.environ.get("MAX_CH", "9999"))

    f32 = mybir.dt.float32
    bf16 = mybir.dt.bfloat16
    i16 = mybir.dt.int16
    i32 = mybir.dt.int32
    AG = "AllGather"
    BYP = mybir.AluOpType.bypass
    RG = [list(range(N_CORES))]

    nc = bacc.Bacc("TRN2", target_bir_lowering=False, debug=False,
                   num_devices=N_CORES, dynamic_dma_scratch_size=32768)

    # ---- inputs ----
    xTs = nc.dram_tensor("xTs", [IN_CH, SH], bf16, kind="ExternalInput")
    W1 = nc.dram_tensor("W1", [IN_CH, 520], bf16, kind="ExternalInput")
    W2 = nc.dram_tensor("W2", [OUT1, 520], bf16, kind="ExternalInput")
    bias1 = nc.dram_tensor("bias1", [128, 512], f32, kind="ExternalInput")
    bias2 = nc.dram_tensor("bias2", [128, 512], f32, kind="ExternalInput")
    gixc = nc.dram_tensor("gixc", [16, CONV_E // 16], i16, kind="ExternalInput")
    gixd = nc.dram_tensor("gixd", [16, CONV_E // 16], i16, kind="ExternalInput")
    sixc = nc.dram_tensor("sixc", [16, CONV_T * 8], i16, kind="ExternalInput")
    sofc = nc.dram_tensor("sofc", [128, CONV_T], f32, kind="ExternalInput")
    gixl = nc.dram_tensor("gixl", [16, LP_E // 16], i16, kind="ExternalInput")
    sixl = nc.dram_tensor("sixl", [16, LP_T * 8], i16, kind="ExternalInput")
    sofl = nc.dram_tensor("sofl", [128, LP_T], f32, kind="ExternalInput")
    disd = nc.dram_tensor("disd", [128, LT], f32, kind="ExternalInput")
    dishd = nc.dram_tensor("dishd", [128, LT], f32, kind="ExternalInput")
    batchd = nc.dram_tensor("batchd", [128, LT], f32, kind="ExternalInput")

    # ---- internal DRAM ----
    H1ps = nc.dram_tensor("H1ps", [SH, COLS], bf16)
    H1p = nc.dram_tensor("H1p", [NP, COLS], bf16)
    H2ps = nc.dram_tensor("H2ps", [SH, COLS], bf16)
    H2p = nc.dram_tensor("H2p", [NP, COLS], bf16)
    OUTC1 = nc.dram_tensor("OUTC1", [NLOC_PAD, COLS], bf16)
    OUTC2 = nc.dram_tensor("OUTC2", [NLOC_PAD, COLS], bf16)
    y0s = nc.dram_tensor("y0s", [SH, 512], bf16)
    z0s = nc.dram_tensor("z0s", [SH, 512], bf16)
    y2s = nc.dram_tensor("y2s", [SH, 512], bf16)
    z2s = nc.dram_tensor("z2s", [SH, 512], bf16)
    shp = [nc.dram_tensor(f"sh{i}", [SH, 512], bf16) for i in range(4)]
    Yt = [nc.dram_tensor(f"Y{i}", [NP, 512], bf16) for i in range(4)]
    OUTL = [nc.dram_tensor(f"OUTL{i}", [NLOC_PAD, 512], bf16) for i in range(4)]
    pooled = nc.dram_tensor("pooled", [64, 1024], f32, kind="ExternalOutput")
    dbg = os.environ.get("DBG", "0") == "1"
    dbg_t = {}
    if dbg:
        for nm, (rr, cc) in dict(
                dH1ps=(SH, COLS), dOUTC1=(NLOC_PAD, COLS), dy0s=(SH, 512),
                dsh0=(SH, 512), dOUTL0=(NLOC_PAD, 512), dsh1=(SH, 512),
                dy2s=(SH, 512), dH2ps=(SH, COLS), dOUTC2=(NLOC_PAD, COLS),
                dz0s=(SH, 512), dz2s=(SH, 512)).items():
            dbg_t[nm] = nc.dram_tensor(nm, [rr, cc], bf16,
                                       kind="ExternalOutput")

    with tile.TileContext(nc) as tc, ExitStack() as ctx:
        st = ctx.enter_context(tc.tile_pool(name="st", bufs=1))
        mybir_ = mybir

        # ---------- static SBUF ----------
        iota_i = st.tile([128, 128], i32, tag="iota_i")
        nc.gpsimd.iota(iota_i[:], [[1, 128]], channel_multiplier=0)
        iota_f = st.tile([128, 128], f32, tag="iota_f")
        nc.vector.tensor_copy(iota_f[:], iota_i[:])
        iota_pm = st.tile([128, 128], i32, tag="iota_pm")
        nc.gpsimd.iota(iota_pm[:], [[1, 128]], channel_multiplier=-1)
        ident = st.tile([128, 128], bf16, tag="ident")
        nc.vector.tensor_single_scalar(ident[:], iota_pm[:], 0,
                                       mybir_.AluOpType.is_equal)
        iota64_f = st.tile([128, 64], f32, tag="iota64_f")
        nc.vector.tensor_copy(iota64_f[:], iota_i[:, 0:64])

        def load_const(t_dram, shape, dt, tg):
            t = st.tile(shape, dt, tag=tg, name=tg)
            nc.sync.dma_start(t[:], t_dram[:, :])
            return t

        def load_idx(t_dram, ncols, tg):
            t = st.tile([128, ncols], i16, tag=tg, name=tg)
            for k in range(8):
                nc.sync.dma_start(t[16 * k:16 * (k + 1), :], t_dram[:, :])
            return t

        W1s = st.tile([128, 2, 520], bf16, tag="W1s")
        nc.sync.dma_start(W1s[:, 0, :], W1[0:128, :])
        nc.sync.dma_start(W1s[:, 1, :], W1[128:256, :])
        W2s = st.tile([128, 4, 520], bf16, tag="W2s")
        for k in range(4):
            nc.sync.dma_start(W2s[:, k, :], W2[k * 128:(k + 1) * 128, :])
        b1s = load_const(bias1, [128, 512], f32, "b1s")
        b2s = load_const(bias2, [128, 512], f32, "b2s")
        gixc_s = load_idx(gixc, CONV_E // 16, "gixc_s")
        gixd_s = load_idx(gixd, CONV_E // 16, "gixd_s")
        sixc_s = load_idx(sixc, CONV_T * 8, "sixc_s")
        sofc_s = load_const(sofc, [128, CONV_T], f32, "sofc_s")
        gixl_s = load_idx(gixl, LP_E // 16, "gixl_s")
        sixl_s = load_idx(sixl, LP_T * 8, "sixl_s")
        sofl_s = load_const(sofl, [128, LP_T], f32, "sofl_s")
        dis_s = load_const(disd, [128, LT], f32, "dis_s")
        dish_s = load_const(dishd, [128, LT], f32, "dish_s")
        batch_s = load_const(batchd, [128, LT], f32, "batch_s")
        y2T_sb = st.tile([128, 4, SH], bf16, tag="y2T_sb")

        zero640 = st.tile([128, COLS], bf16, tag="zero640")
        nc.vector.memset(zero640[:], 0.0)

        # ---------- helpers ----------
        def gemm(pool, psp, src_lhsT, Wsb, nk, out_dram):
            """node-sharded GEMM: out[t*128+p, 0:520] = sum_k lhsT_k.T @ W."""
            for t in range(LT):
                psA = psp.tile([128, 512], f32, space="PSUM", tag="psA",
                               bufs=2)
                psB = psp.tile([128, 8], f32, space="PSUM", tag="psB", bufs=2)
                lts = [src_lhsT(pool, t, k) for k in range(nk)]
                for k in range(nk):
                    nc.tensor.matmul(psA[:], lhsT=lts[k], rhs=Wsb[:, k, 0:512],
                                     start=(k == 0), stop=(k == nk - 1))
                for k in range(nk):
                    nc.tensor.matmul(psB[:], lhsT=lts[k], rhs=Wsb[:, k, 512:520],
                                     start=(k == 0), stop=(k == nk - 1))
                ht = pool.tile([128, COLS], bf16, tag="gemm_out")
                nc.vector.tensor_copy(ht[:, 0:512], psA[:])
                nc.scalar.copy(ht[:, 512:520], psB[:])
                nc.sync.dma_start(out_dram[t * 128:(t + 1) * 128, :], ht[:])

        def zero_out(out_dram, width):
            for t in range(NLOC_PAD // 128):
                nc.sync.dma_start(out_dram[t * 128:(t + 1) * 128, :],
                                  zero640[:, 0:width])
            # remaining 64 rows
            nc.sync.dma_start(out_dram[SH:SH + 64, :], zero640[0:64, 0:width])

        def conv_edges(pool, psp, Htab, outd, maxk):
            """conv edge phase: gather/logits/segsum-matmul/scatter."""
            for ch in range(min(CONV_CHUNKS, max_ch)):
                TT = CHUNK_C // 128  # 16
                IW = CHUNK_C // 16   # idx cols per chunk
                G = pool.tile([128, TT, COLS], bf16, tag="G")
                ED = pool.tile([128, TT, 128], bf16, tag="ED")
                if no_gath:
                    nc.vector.memset(G[:], 0.25)
                    nc.vector.memset(ED[:], 0.25)
                else:
                    ns = CHUNK_C // 1024  # sub-gathers of 1024 idxs
                    for s_ in range(ns):
                        i0 = ch * IW + s_ * 64
                        t0 = s_ * 8
                        nc.gpsimd.dma_gather(
                            G[:, t0:t0 + 8, :], Htab[:, :],
                            gixc_s[:, i0:i0 + 64], 1024, 1024, COLS)
                        nc.gpsimd.dma_gather(
                            ED[:, t0:t0 + 8, :], Htab[:, 512:640],
                            gixd_s[:, i0:i0 + 64], 1024, 1024, 128,
                            elem_step=COLS)
                Ef = pool.tile([128, TT, HEADS], f32, tag="Ef")
                nc.vector.tensor_tensor(Ef[:], G[:, :, 512:516], ED[:, :, 4:8],
                                        mybir_.AluOpType.add)
                El = pool.tile([128, TT, HEADS], f32, tag="El")
                nc.vector.scalar_tensor_tensor(El[:], Ef[:], NEG, Ef[:],
                                               mybir_.AluOpType.mult,
                                               mybir_.AluOpType.max)
                Ab = pool.tile([128, TT, HEADS], bf16, tag="Ab")
                nc.scalar.activation(Ab[:], El[:],
                                     mybir_.ActivationFunctionType.Exp)
                nc.vector.memset(G[:, :, 512:516], 1.0)
                so = sofc_s[:, ch * TT:(ch + 1) * TT]
                I = pool.tile([128, TT, 128], bf16, tag="I")
                nc.vector.tensor_tensor(
                    I[:],
                    iota_f[:].unsqueeze(1).broadcast_to([128, TT, 128]),
                    so.unsqueeze(2).broadcast_to([128, TT, 128]),
                    mybir_.AluOpType.is_equal)
                for tt in range(TT):
                    t = ch * TT + tt
                    S = pool.tile([128, HEADS, 128], bf16, tag="S", bufs=4)
                    nc.vector.tensor_tensor(
                        S[:],
                        I[:, tt, :].unsqueeze(1).broadcast_to([128, HEADS, 128]),
                        Ab[:, tt, :].unsqueeze(2).broadcast_to([128, HEADS, 128]),
                        mybir_.AluOpType.mult)
                    scat = pool.tile([128, COLS], bf16, tag="scat", bufs=4)
                    for h in range(HEADS):
                        acc = psp.tile([128, 129], f32, space="PSUM", tag="accC", bufs=8)
                        nc.tensor.matmul(acc[:, 0:128], lhsT=S[:, h, :],
                                         rhs=G[:, tt, h * 128:(h + 1) * 128],
                                         start=True, stop=True)
                        nc.tensor.matmul(acc[:, 128:129], lhsT=S[:, h, :],
                                         rhs=G[:, tt, 512:513],
                                         start=True, stop=True)
                        if h % 2 == 0:
                            nc.vector.tensor_copy(
                                scat[:, h * 128:(h + 1) * 128], acc[:, 0:128])
                            nc.vector.tensor_copy(
                                scat[:, 512 + h:513 + h], acc[:, 128:129])
                        else:
                            nc.scalar.copy(
                                scat[:, h * 128:(h + 1) * 128], acc[:, 0:128])
                            nc.scalar.copy(
                                scat[:, 512 + h:513 + h], acc[:, 128:129])
                    if not no_scat:
                        nc.gpsimd.dma_scatter_add(
                            outd[:, :], scat[:].unsqueeze(1),
                            sixc_s[:, t * 8:(t + 1) * 8],
                            128, int(maxk[t]), COLS)
                    else:
                        nc.sync.dma_start(
                            outd[0:128, :], scat[:])

        def conv_div(pool, psp, outd, bsb, ysd, ypd, stash_T):
            """softmax divide + bias + relu; write row shard + dis-scaled shard;
            optionally stash transposed into y2T_sb."""
            for t in range(LT):
                ot = pool.tile([128, COLS], bf16, tag="ot")
                nc.sync.dma_start(ot[:], outd[t * 128:(t + 1) * 128, :])
                d4 = pool.tile([128, 4], f32, tag="d4")
                nc.vector.tensor_scalar_add(d4[:], ot[:, 512:516], EPS)
                dr = pool.tile([128, 4], f32, tag="dr")
                nc.vector.reciprocal(dr[:], d4[:])
                otf = pool.tile([128, 512], f32, tag="otf")
                nc.scalar.copy(otf[:], ot[:, 0:512])
                ym = pool.tile([128, HEADS, 128], f32, tag="ym")
                nc.vector.tensor_tensor(
                    ym[:],
                    otf[:].rearrange("p (h c) -> p h c", h=HEADS),
                    dr[:].unsqueeze(2).broadcast_to([128, HEADS, 128]),
                    mybir_.AluOpType.mult)
                yb = pool.tile([128, 512], f32, tag="yb")
                nc.vector.tensor_add(yb[:], ym[:].rearrange("p h c -> p (h c)"),
                                     bsb[:])
                yrow = pool.tile([128, 512], bf16, tag="yrow")
                nc.vector.tensor_scalar_max(yrow[:], yb[:], 0.0)
                nc.sync.dma_start(ysd[t * 128:(t + 1) * 128, :], yrow[:])
                ysc = pool.tile([128, 512], bf16, tag="ysc")
                nc.vector.tensor_scalar(ysc[:], yb[:], 0.0, dis_s[:, t:t + 1],
                                        mybir_.AluOpType.max,
                                        mybir_.AluOpType.mult)
                nc.sync.dma_start(ypd[t * 128:(t + 1) * 128, :], ysc[:])
                if stash_T:
                    for k in range(4):
                        pt = psp.tile([128, 128], bf16, space="PSUM", tag="ptT", bufs=2)
                        nc.tensor.transpose(pt[:], yrow[:, k * 128:(k + 1) * 128],
                                            ident[:])
                        nc.scalar.copy(y2T_sb[:, k, t * 128:(t + 1) * 128], pt[:])

        def lp_edges(pool, psp, Ytab, outd, maxk):
            for ch in range(LP_CHUNKS):
                TT = CHUNK_L // 128  # 32
                IW = CHUNK_L // 16
                G = pool.tile([128, TT, 512], bf16, tag="GL")
                for s_ in range(CHUNK_L // 1024):
                    i0 = ch * IW + s_ * 64
                    t0 = s_ * 8
                    nc.gpsimd.dma_gather(
                        G[:, t0:t0 + 8, :], Ytab[:, :],
                        gixl_s[:, i0:i0 + 64], 1024, 1024, 512)
                so = sofl_s[:, ch * TT:(ch + 1) * TT]
                I = pool.tile([128, TT, 128], bf16, tag="IL")
                nc.vector.tensor_tensor(
                    I[:],
                    iota_f[:].unsqueeze(1).broadcast_to([128, TT, 128]),
                    so.unsqueeze(2).broadcast_to([128, TT, 128]),
                    mybir_.AluOpType.is_equal)
                for tt in range(TT):
                    t = ch * TT + tt
                    acc = psp.tile([128, 512], f32, space="PSUM", tag="accL", bufs=4)
                    nc.tensor.matmul(acc[:], lhsT=I[:, tt, :], rhs=G[:, tt, :],
                                     start=True, stop=True)
                    scat = pool.tile([128, 512], bf16, tag="scatL", bufs=4)
                    if tt % 2 == 0:
                        nc.vector.tensor_copy(scat[:], acc[:])
                    else:
                        nc.scalar.copy(scat[:], acc[:])
                    nc.gpsimd.dma_scatter_add(
                        outd[:, :], scat[:].unsqueeze(1),
                        sixl_s[:, t * 8:(t + 1) * 8],
                        128, int(maxk[t]), 512)

        def lp_div(pool, psp, outd, resd, ypd, yrow_d, stash_T):
            """y = clip(dis*0.5*agg + 0.5*res, 0, 1); write scaled shard and
            optionally row shard / transposed stash."""
            for t in range(LT):
                ot = pool.tile([128, 512], bf16, tag="lot")
                nc.sync.dma_start(ot[:], outd[t * 128:(t + 1) * 128, :])
                rt = pool.tile([128, 512], bf16, tag="lrt")
                nc.sync.dma_start(rt[:], resd[t * 128:(t + 1) * 128, :])
                t1 = pool.tile([128, 512], f32, tag="lt1")
                nc.vector.tensor_scalar_mul(t1[:], ot[:], dish_s[:, t:t + 1])
                rtf = pool.tile([128, 512], f32, tag="lrtf")
                nc.scalar.mul(rtf[:], rt[:], 0.5)
                t2 = pool.tile([128, 512], f32, tag="lt2")
                nc.vector.tensor_add(t2[:], rtf[:], t1[:])
                yrow = pool.tile([128, 512], bf16, tag="lyrow")
                nc.vector.tensor_scalar(yrow[:], t2[:], 0.0, 1.0,
                                        mybir_.AluOpType.max,
                                        mybir_.AluOpType.min)
                if yrow_d is not None:
                    nc.sync.dma_start(yrow_d[t * 128:(t + 1) * 128, :], yrow[:])
                if ypd is not None:
                    ysc = pool.tile([128, 512], bf16, tag="lysc")
                    nc.vector.tensor_scalar_mul(ysc[:], yrow[:],
                                                dis_s[:, t:t + 1])
                    nc.sync.dma_start(ypd[t * 128:(t + 1) * 128, :], ysc[:])
                if stash_T:
                    for k in range(4):
                        pt = psp.tile([128, 128], bf16, space="PSUM", tag="ptT", bufs=2)
                        nc.tensor.transpose(pt[:], yrow[:, k * 128:(k + 1) * 128],
                                            ident[:])
                        nc.scalar.copy(y2T_sb[:, k, t * 128:(t + 1) * 128],
                                       pt[:])

        def allgather(shard_d, full_d):
            nc.gpsimd.collective_compute(AG, BYP, replica_groups=RG,
                                         ins=[shard_d[:, :]],
                                         outs=[full_d[:, :]])

        # ================= pipeline =================
        if phases >= 1:
            with tc.tile_pool(name="g1", bufs=2) as pool, \
                 tc.tile_pool(name="g1p", bufs=4, space="PSUM") as psp:
                def x_lhsT(pool, t, k):
                    lt = pool.tile([128, 128], bf16, tag="xlt", bufs=4)
                    nc.sync.dma_start(
                        lt[:], xTs[k * 128:(k + 1) * 128, t * 128:(t + 1) * 128])
                    return lt[:]
                gemm(pool, psp, x_lhsT, W1s, 2, H1ps)
        if phases >= 2:
            allgather(H1ps, H1p)

        if phases >= 3:
            with tc.tile_pool(name="c1", bufs=2) as pool, \
                 tc.tile_pool(name="c1p", bufs=8, space="PSUM") as psp:
                zero_out(OUTC1, COLS)
                conv_edges(pool, psp, H1p, OUTC1, conv_maxk)
                conv_div(pool, psp, OUTC1, b1s, y0s, shp[0], False)
        if phases >= 4:
            allgather(shp[0], Yt[0])

        if phases >= 5:
            with tc.tile_pool(name="l1", bufs=2) as pool, \
                 tc.tile_pool(name="l1p", bufs=4, space="PSUM") as psp:
                zero_out(OUTL[0], 512)
                lp_edges(pool, psp, Yt[0], OUTL[0], lp_maxk)
                lp_div(pool, psp, OUTL[0], y0s, shp[1], None, False)
        if phases >= 6:
            allgather(shp[1], Yt[1])
            with tc.tile_pool(name="l2", bufs=2) as pool, \
                 tc.tile_pool(name="l2p", bufs=4, space="PSUM") as psp:
                zero_out(OUTL[1], 512)
                lp_edges(pool, psp, Yt[1], OUTL[1], lp_maxk)
                lp_div(pool, psp, OUTL[1], y0s, None, y2s, True)

        if phases >= 7:
            with tc.tile_pool(name="g2", bufs=2) as pool, \
                 tc.tile_pool(name="g2p", bufs=4, space="PSUM") as psp:
                def y_lhsT(pool, t, k):
                    return y2T_sb[:, k, t * 128:(t + 1) * 128]
                gemm(pool, psp, y_lhsT, W2s, 4, H2ps)
            allgather(H2ps, H2p)

        if phases >= 8:
            with tc.tile_pool(name="c2", bufs=2) as pool, \
                 tc.tile_pool(name="c2p", bufs=8, space="PSUM") as psp:
                zero_out(OUTC2, COLS)
                conv_edges(pool, psp, H2p, OUTC2, conv_maxk)
                conv_div(pool, psp, OUTC2, b2s, z0s, shp[2], False)
            allgather(shp[2], Yt[2])

        if phases >= 9:
            with tc.tile_pool(name="l3", bufs=2) as pool, \
                 tc.tile_pool(name="l3p", bufs=4, space="PSUM") as psp:
                zero_out(OUTL[2], 512)
                lp_edges(pool, psp, Yt[2], OUTL[2], lp_maxk)
                lp_div(pool, psp, OUTL[2], z0s, shp[3], None, False)
            allgather(shp[3], Yt[3])
            with tc.tile_pool(name="l4", bufs=2) as pool, \
                 tc.tile_pool(name="l4p", bufs=4, space="PSUM") as psp:
                zero_out(OUTL[3], 512)
                lp_edges(pool, psp, Yt[3], OUTL[3], lp_maxk)
                lp_div(pool, psp, OUTL[3], z0s, None, z2s, False)

        if dbg:
            with tc.tile_pool(name="dbgp", bufs=2) as pool:
                pairs = [("dH1ps", H1ps, SH, COLS), ("dOUTC1", OUTC1, NLOC_PAD, COLS),
                         ("dy0s", y0s, SH, 512), ("dsh0", shp[0], SH, 512),
                         ("dOUTL0", OUTL[0], NLOC_PAD, 512), ("dsh1", shp[1], SH, 512),
                         ("dy2s", y2s, SH, 512), ("dH2ps", H2ps, SH, COLS),
                         ("dOUTC2", OUTC2, NLOC_PAD, COLS), ("dz0s", z0s, SH, 512),
                         ("dz2s", z2s, SH, 512)]
                for nm, ten, rr, cc in pairs:
                    if nm not in dbg_t:
                        continue
                    full = NLOC_PAD if rr == NLOC_PAD else SH
                    nt = full // 128
                    for t in range(nt):
                        ct = pool.tile([128, cc], bf16, tag="dbgt")
                        nc.sync.dma_start(ct[:], ten[t * 128:(t + 1) * 128, 0:cc])
                        nc.sync.dma_start(dbg_t[nm][t * 128:(t + 1) * 128, :], ct[:])
                    if rr == NLOC_PAD:
                        ct = pool.tile([128, cc], bf16, tag="dbgt")
                        nc.sync.dma_start(ct[0:64, :], ten[SH:SH + 64, 0:cc])
                        nc.sync.dma_start(dbg_t[nm][SH:SH + 64, :], ct[0:64, :])

        # ---- pooling ----
        with tc.tile_pool(name="pl", bufs=2) as pool, \
             tc.tile_pool(name="plp", bufs=2, space="PSUM") as psp:
            po = pool.tile([64, 1024], f32, tag="po")
            if phases >= 10:
                ps1 = psp.tile([64, 512], f32, space="PSUM", tag="ps1", bufs=1)
                ps2 = psp.tile([64, 512], f32, space="PSUM", tag="ps2", bufs=1)
                for t in range(LT):
                    IB = pool.tile([128, 64], bf16, tag="IB")
                    nc.vector.tensor_scalar(IB[:], iota64_f[:],
                                            batch_s[:, t:t + 1],
                                            None, mybir_.AluOpType.is_equal)
                    yt = pool.tile([128, 512], bf16, tag="pyt")
                    nc.sync.dma_start(yt[:], y2s[t * 128:(t + 1) * 128, :])
                    zt = pool.tile([128, 512], bf16, tag="pzt")
                    nc.sync.dma_start(zt[:], z2s[t * 128:(t + 1) * 128, :])
                    nc.tensor.matmul(ps1[:], lhsT=IB[:], rhs=yt[:],
                                     start=(t == 0), stop=(t == LT - 1))
                    nc.tensor.matmul(ps2[:], lhsT=IB[:], rhs=zt[:],
                                     start=(t == 0), stop=(t == LT - 1))
                nc.vector.tensor_copy(po[:, 0:512], ps1[:])
                nc.vector.tensor_copy(po[:, 512:1024], ps2[:])
            else:
                nc.vector.memset(po[:], 0.0)
            nc.sync.dma_start(pooled[:, :], po[:])

    nc.compile()
    return nc


# ---------------- entry ----------------

def _make_runner(nc):
    """Cached jitted shard_map runner (run_bass_via_pjrt rebuilds its jit
    every call -> re-trace + XLA compile each time; we build once)."""
    import jax
    import numpy as _np
    from jax.experimental.shard_map import shard_map
    from jax.sharding import Mesh, PartitionSpec
    from concourse import mybir
    from concourse.bass2jax import (_bass_exec_p, partition_id_tensor,
                                    install_neuronx_cc_hook)
    install_neuronx_cc_hook()

    partition_name = (nc.partition_id_tensor.name
                      if nc.partition_id_tensor else None)
    in_names, out_names, out_avals, zero_shapes = [], [], [], []
    for alloc in nc.m.functions[0].allocations:
        if not isinstance(alloc, mybir.MemoryLocationSet):
            continue
        name = alloc.memorylocations[0].name
        if alloc.kind == "ExternalInput":
            if name != partition_name:
                in_names.append(name)
        elif alloc.kind == "ExternalOutput":
            out_names.append(name)
            shape = tuple(alloc.tensor_shape)
            dtype = mybir.dt.np(alloc.dtype)
            out_avals.append(jax.core.ShapedArray(shape, dtype))
            zero_shapes.append((shape, dtype))
    n_params = len(in_names)
    n_outs = len(out_avals)
    all_in = list(in_names) + list(out_names)
    if partition_name is not None:
        all_in.append(partition_name)
    donate = tuple(range(n_params, n_params + n_outs))

    def _body(*args):
        operands = list(args)
        if partition_name is not None:
            operands.append(partition_id_tensor())
        outs = _bass_exec_p.bind(
            *operands,
            out_avals=tuple(out_avals),
            in_names=tuple(all_in),
            out_names=tuple(out_names),
            lowering_input_output_aliases=(),
            sim_require_finite=True,
            sim_require_nnan=True,
            nc=nc,
        )
        return tuple(outs)

    devices = jax.devices()[:N_CORES]
    mesh = Mesh(_np.asarray(devices), ("core",))
    in_specs = (PartitionSpec("core"),) * (n_params + n_outs)
    out_specs = (PartitionSpec("core"),) * n_outs
    sharded = jax.jit(
        shard_map(_body, mesh=mesh, in_specs=in_specs, out_specs=out_specs,
                  check_rep=False),
        donate_argnums=donate, keep_unused=True)

    def run(in_maps):
        per_core = [[_np.asarray(m[name]) for name in in_names]
                    for m in in_maps]
        concat_in = [
            _np.concatenate([per_core[c][i] for c in range(N_CORES)], axis=0)
            for i in range(n_params)]
        concat_zeros = [
            _np.zeros((N_CORES * s[0], *s[1:]), d) for (s, d) in zero_shapes]
        out_arrs = sharded(*concat_in, *concat_zeros)
        return [
            {name: _np.asarray(out_arrs[i]).reshape(
                N_CORES, *out_avals[i].shape)[c]
             for i, name in enumerate(out_names)}
            for c in range(N_CORES)]

    return run


class _Res:
    def __init__(self, results):
        self.results = results
        self.exec_time_ns = None


def kernel(x, edge_index, batch,
           conv1_W, conv1_asrc, conv1_adst, conv1_b,
           conv2_W, conv2_asrc, conv2_adst, conv2_b,
           mlp_W1, mlp_b1, mlp_W2, mlp_b2):

    import os
    import time as _t
    _ts = _t.time()
    x = np.asarray(x, dtype=np.float32)
    batch_np = np.asarray(batch, dtype=np.int64)

    per_core, conv_maxk, lp_maxk, xp, dis = _preprocess(x, edge_index, batch)
    if os.environ.get("KTIME"):
        print(f"[ktime] preprocess: {(_t.time()-_ts)*1e3:.0f} ms")
    _ts = _t.time()

    W1a = _fold_w(conv1_W, conv1_asrc, conv1_adst)
    W2a = _fold_w(conv2_W, conv2_asrc, conv2_adst)
    b1r = np.tile(np.asarray(conv1_b, np.float32)[None, :], (128, 1))
    b2r = np.tile(np.asarray(conv2_b, np.float32)[None, :], (128, 1))

    xT = np.ascontiguousarray(xp.T).astype(BF16)  # [256, NP]

    in_maps = []
    for c in range(N_CORES):
        pc = per_core[c]
        in_maps.append({
            "xTs": np.ascontiguousarray(xT[:, c * SH:(c + 1) * SH]),
            "W1": W1a, "W2": W2a, "bias1": b1r, "bias2": b2r,
            "gixc": pc["conv"]["gidx_w"], "gixd": pc["conv"]["didx_w"],
            "sixc": pc["conv"]["sidx_w"],
            "sofc": pc["conv"]["segoff_w"],
            "gixl": pc["lp"]["gidx_w"], "sixl": pc["lp"]["sidx_w"],
            "sofl": pc["lp"]["segoff_w"],
            "disd": pc["dis"], "dishd": pc["dish"], "batchd": pc["batch"],
        })

    import os
    phases = int(os.environ.get("PHASES", "10"))
    if os.environ.get("KTIME"):
        print(f"[ktime] in_maps: {(_t.time()-_ts)*1e3:.0f} ms")
    key = (f"nc{phases}-{os.environ.get('NO_SCAT')}-"
           f"{os.environ.get('NO_GATH')}-{os.environ.get('MAX_CH')}-"
           f"{os.environ.get('DBG')}")
    if key not in _cached:
        _cached[key] = _build(conv_maxk, lp_maxk, phases)
    nc = _cached[key]

    rkey = key + "-runner"
    if rkey not in _cached:
        _cached[rkey] = _make_runner(nc)
    import time
    t0 = time.time()
    res = _Res(_cached[rkey](in_maps))
    _cached["device_wall_ns"] = int((time.time() - t0) * 1e9)
    _cached["last_result"] = res
    if os.environ.get("KTIME"):
        print(f"[ktime] device run: {_cached['device_wall_ns'] / 1e6:.1f} ms")

    _ts = _t.time()
    # ---- host postprocess ----
    pool_dev = np.zeros((NGR, 1024), dtype=np.float32)
    for c in range(N_CORES):
        pool_dev += np.asarray(res.results[c]["pooled"]).astype(np.float32)

    b = batch_np
    # batch is sorted: segment-sum via reduceat
    starts = np.searchsorted(b, np.arange(NGR))
    xs = np.add.reduceat(x, starts, axis=0)
    counts_b = np.diff(np.append(starts, len(b)))
    xsums = np.where((counts_b > 0)[:, None], xs, 0.0).astype(np.float32)
    cnts = np.bincount(b, minlength=NGR).astype(np.float32)
    cdiv = np.maximum(cnts, 1.0)[:, None]

    pooled_full = np.concatenate(
        [xsums, pool_dev[:, 0:512], pool_dev[:, 512:1024]], axis=1) / cdiv
    hdd = np.maximum(pooled_full @ np.asarray(mlp_W1, np.float32)
                     + np.asarray(mlp_b1, np.float32), 0.0)
    out = hdd @ np.asarray(mlp_W2, np.float32) + np.asarray(mlp_b2, np.float32)
    if os.environ.get("KTIME"):
        print(f"[ktime] postprocess: {(_t.time()-_ts)*1e3:.0f} ms")
    return out.astype(np.float32)
